# revision 11
# baseline (speedup 1.0000x reference)
"""GQA attention (32 q heads / 8 kv heads, D=64, HID=2048, B=2, T=2048)
distributed over 8 TRN2 NeuronCores.

Sharding: 2-way data parallel (batch) x 4-way tensor parallel (head groups).
Core c handles batch c//4 and head group g=c%4 (q heads [8g,8g+8), kv heads
[2g,2g+2)).  Each core projects Q^T/K^T (transposed layout: head-dims on
partitions, T on free axis), computes V^T the same cheap way (weights
stationary, N=512 streams) and PE-transposes it into the [keys, dims] layout
that P@V needs.  Scores^T = K @ Q^T per head with keys on partitions, exp via
ScalarE (no max-subtraction needed at these magnitudes; masked entries
multiply to exactly 0 by a host-precomputed exp(mask) factor), then
out^T = Vext^T @ P^T where Vext carries a ones column producing the softmax
denominators for free.

The attention phase is ScalarE(exp)-bound (~1.33us per key-chunk vs ~0.78us
of matmul), so all other PE work — next block's projections, o_proj of
previous q-tiles, gather loads — is chopped into ~1us micro-steps and
interleaved ONE PER KEY-CHUNK into the attention emission, with the score
matmuls software-pipelined one chunk ahead so the exp stream never waits.
Attention outputs are AllGathered per head-pair (16 gathers of 128KB); a
dummy gather at t=0 absorbs the collective warm-up, and the last q-tile's
o_proj accumulates m-major so only its final quarter waits on the last
gather.  All host-side layouts give every DMA >=4KB contiguous per partition
(hardware DGE fast path).
"""

import os
import numpy as np
import ml_dtypes

BF16 = ml_dtypes.bfloat16

HQ, HKV, D, HID, THETA = 32, 8, 64, 2048, 10000.0
NCORES, NGROUPS = 8, 4
QDIM = HQ * D // NGROUPS        # 512 q dims per core
KVDIM = HKV * D // NGROUPS      # 128 kv dims per core
NQT = 512                       # query tile (free dim per PSUM bank)
NKC = 128                       # key chunk (partition dim)

_cache = {}
LAST_RESULT = None              # BassKernelResults of the most recent run


def plan_mask(mask, T):
    """Classify (key-chunk i, q-tile j) tiles of exp(mask).T.

    Returns (plans, emt_tiles): plans[j] = list of (i, kind, emt_idx) where
    kind 0 = no mask needed (exp(mask)==1 on tile), kind 1 = multiply by
    emt_tiles[emt_idx].  All-zero tiles are skipped entirely (they contribute
    nothing to P@V nor to the softmax denominator).
    """
    m = np.asarray(mask, dtype=np.float32).reshape(T, T)
    with np.errstate(under="ignore"):
        em = np.exp(m).T.astype(np.float32)   # em[k, q] = exp(mask[q, k])
    nj, nk = T // NQT, T // NKC
    plans, emt_tiles = [], []
    for j in range(nj):
        pj = []
        for i in range(nk):
            t = em[i * NKC:(i + 1) * NKC, j * NQT:(j + 1) * NQT]
            if not t.any():
                continue
            if (t == 1.0).all():
                pj.append((i, 0, -1))
            else:
                pj.append((i, 1, len(emt_tiles)))
                emt_tiles.append(t.astype(BF16))
        plans.append(pj)
    return plans, emt_tiles


def build_graph(T, plans, n_emt):
    """Build the SPMD Bacc graph (same on all 8 cores; shards arrive as data)."""
    import concourse.bass as bass  # noqa: F401
    import concourse.mybir as mybir
    import concourse.tile as tile
    from concourse import bacc, masks

    f32, bf16 = mybir.dt.float32, mybir.dt.bfloat16
    AF, ALU = mybir.ActivationFunctionType, mybir.AluOpType

    nj = T // NQT          # q tiles
    nhc = HID // 128       # contraction chunks over hidden dim (16)
    noc = (HQ * D) // 128  # contraction chunks over gathered head dim (16)
    nem = max(n_emt, 1)
    assert nj == 4, "schedule below is specialized for T=2048"

    nc = bacc.Bacc("TRN2", target_bir_lowering=False, debug=False,
                   num_devices=NCORES)

    # host-prepped layouts: every tensor reads contiguous >=4KB per partition
    xt = nc.dram_tensor("xt", [nj, 128, nhc, NQT], bf16,
                        kind="ExternalInput").ap()
    wqt = nc.dram_tensor("wqt", [128, nhc, QDIM], bf16,
                         kind="ExternalInput").ap()
    wkt = nc.dram_tensor("wkt", [128, nhc, KVDIM], bf16,
                         kind="ExternalInput").ap()
    wvt = nc.dram_tensor("wvt", [128, nhc, KVDIM], bf16,
                         kind="ExternalInput").ap()
    qb = nc.dram_tensor("qb", [128, 4], f32, kind="ExternalInput").ap()
    kb = nc.dram_tensor("kb", [128, 1], f32, kind="ExternalInput").ap()
    vb = nc.dram_tensor("vb", [1, KVDIM], bf16, kind="ExternalInput").ap()
    cosq = nc.dram_tensor("cosq", [128, T], f32, kind="ExternalInput").ap()
    ssin = nc.dram_tensor("ssin", [128, T], f32, kind="ExternalInput").ap()
    emt = nc.dram_tensor("emt", [128, nem, NQT], bf16,
                         kind="ExternalInput").ap()
    owt = nc.dram_tensor("owt", [128, noc, QDIM], bf16,
                         kind="ExternalInput").ap()
    out = nc.dram_tensor("out", [T, QDIM], bf16, kind="ExternalOutput").ap()

    rg = [[0, 1, 2, 3], [4, 5, 6, 7]]

    with tile.TileContext(nc) as tc:
        with tc.tile_pool(name="dramp", bufs=1, space="DRAM") as dramp:
            ag_in = [[dramp.tile([128, NQT], bf16, name=f"agin{j}_{m}")
                      for m in range(4)] for j in range(nj)]
            ag_out = [[dramp.tile([NGROUPS * 128, NQT], bf16,
                                  name=f"agout{j}_{m}")
                       for m in range(4)] for j in range(nj)]
            dum_in = dramp.tile([1, 128], bf16, name="dum_in")
            dum_out = dramp.tile([4, 128], bf16, name="dum_out")

        with tc.tile_pool(name="persist", bufs=1) as pp:
            # Q^T per head-pair chunk: [128 (2 heads x 64), T]
            qt = [pp.tile([128, T], bf16, name=f"qt{m}") for m in range(4)]
            # K^T duplicated per kv head: [128 = kv dup'd twice, T]
            ktd = [pp.tile([128, T], bf16, name=f"ktd{k}") for k in range(2)]
            # V per key chunk: [128 keys, 130] (V0|one|V1|one)
            vsb = [pp.tile([128, 130], bf16, name=f"v{i}")
                   for i in range(T // NKC)]
            ow_sb = pp.tile([128, noc, QDIM], bf16, name="ow_sb")
            wqq = [pp.tile([128, 4, QDIM], bf16, name=f"wqq{r}")
                   for r in range(4)]
            wk_sb = pp.tile([128, nhc, KVDIM], bf16, name="wk_sb")
            wv_sb = pp.tile([128, nhc, KVDIM], bf16, name="wv_sb")
            cos_sb = pp.tile([128, T], f32, name="cos_sb")
            ssin_sb = pp.tile([128, T], f32, name="ssin_sb")
            emt_sb = pp.tile([128, nem, NQT], bf16, name="emt_sb")
            ident = pp.tile([128, 128], bf16, name="ident")
            vb_sb = pp.tile([1, KVDIM], bf16, name="vb_sb")
            ones_row = pp.tile([1, NQT], bf16, name="ones_row")
            qb_sb = pp.tile([128, 4], f32, name="qb_sb")
            kb_sb = pp.tile([128, 1], f32, name="kb_sb")

            nc.sync.dma_start(out=qb_sb[:], in_=qb)
            nc.sync.dma_start(out=kb_sb[:], in_=kb)
            nc.sync.dma_start(out=vb_sb[:], in_=vb)
            nc.vector.memset(ones_row[:], 1.0)
            # dummy gather: absorbs the one-time collective barrier/warm-up
            # (~12us trigger delay + ~2.5x duration) before the first real one
            nc.sync.dma_start(out=dum_in[:], in_=ones_row[0:1, 0:128])
            nc.gpsimd.collective_compute(
                "AllGather", ALU.bypass, replica_groups=rg,
                ins=[dum_in.opt()], outs=[dum_out.opt()])
            masks.make_identity(nc, ident[:])

            with tc.tile_pool(name="projx", bufs=2) as px, \
                 tc.tile_pool(name="projtmp", bufs=2) as ptmp, \
                 tc.tile_pool(name="pmain", bufs=2, space="PSUM") as pmain, \
                 tc.tile_pool(name="psS", bufs=2, space="PSUM") as psS, \
                 tc.tile_pool(name="psO", bufs=1, space="PSUM") as psO, \
                 tc.tile_pool(name="ptp", bufs=3) as ptp, \
                 tc.tile_pool(name="evp", bufs=2) as evp, \
                 tc.tile_pool(name="agp", bufs=8) as agp, \
                 tc.tile_pool(name="outp", bufs=2) as outp:

                x_tiles = {}

                def load_x_quarter(tb, qr):
                    xq = px.tile([128, 4, NQT], bf16, name="x_sb", tag="x_sb",
                                 bufs=8)
                    nc.sync.dma_start(out=xq[:],
                                      in_=xt[tb, :, qr * 4:(qr + 1) * 4, :])
                    x_tiles.setdefault(tb, []).append(xq)

                # startup: x(0) quarters interleaved with wq quarters so the
                # first Q matmuls start after ~1MB instead of ~5MB of DMA.
                for qr in range(4):
                    load_x_quarter(0, qr)
                    nc.sync.dma_start(out=wqq[qr][:],
                                      in_=wqt[:, qr * 4:(qr + 1) * 4, :])
                nc.sync.dma_start(out=wk_sb[:], in_=wkt)
                nc.sync.dma_start(out=wv_sb[:], in_=wvt)
                nc.sync.dma_start(out=cos_sb[:], in_=cosq)
                nc.sync.dma_start(out=ssin_sb[:], in_=ssin)
                nc.sync.dma_start(out=emt_sb[:], in_=emt)
                nc.sync.dma_start(out=ow_sb[:], in_=owt)

                def rope_evict(ps, bias_col, dst, ts):
                    """dst = RoPE(ps + bias) cast to bf16."""
                    t2 = ptmp.tile([128, NQT], f32, name="t2", tag="t2")
                    nc.vector.scalar_tensor_tensor(
                        t2[:], ps[:], bias_col, ssin_sb[:, ts],
                        op0=ALU.add, op1=ALU.mult)
                    t2s = ptmp.tile([128, NQT], f32, name="t2s", tag="t2s")
                    for blk in range(4):
                        sb = blk ^ 1
                        # off the sync queue: keeps it free for attention
                        nc.gpsimd.dma_start(
                            out=t2s[blk * 32:(blk + 1) * 32, :],
                            in_=t2[sb * 32:(sb + 1) * 32, :])
                    t1 = ptmp.tile([128, NQT], f32, name="t1", tag="t1")
                    nc.vector.scalar_tensor_tensor(
                        t1[:], ps[:], bias_col, cos_sb[:, ts],
                        op0=ALU.add, op1=ALU.mult)
                    nc.vector.tensor_add(dst, t1[:], t2s[:])

                def proj_steps(tb):
                    """Projection of T-block tb as ~1us micro-steps."""
                    ts = slice(tb * NQT, (tb + 1) * NQT)
                    state = {}

                    def qs(m, qr):
                        def f():
                            xq = x_tiles[tb]
                            if qr == 0:
                                state[m] = pmain.tile([128, NQT], f32,
                                                      name="ps", tag="ps")
                            ps = state[m]
                            for c in range(qr * 4, qr * 4 + 4):
                                nc.tensor.matmul(
                                    ps[:],
                                    wqq[c // 4][:, c % 4,
                                                m * 128:(m + 1) * 128],
                                    xq[c // 4][:, c % 4, :],
                                    start=(c == 0), stop=(c == nhc - 1))
                            if qr == 3:
                                rope_evict(ps, qb_sb[:, m:m + 1],
                                           qt[m][:, ts], ts)
                        return f

                    def ks(qr):
                        def f():
                            xq = x_tiles[tb]
                            if qr == 0:
                                state['k'] = pmain.tile([128, NQT], f32,
                                                        name="psk", tag="ps")
                            psk = state['k']
                            for c in range(qr * 4, qr * 4 + 4):
                                nc.tensor.matmul(psk[:], wk_sb[:, c, :],
                                                 xq[c // 4][:, c % 4, :],
                                                 start=(c == 0),
                                                 stop=(c == nhc - 1))
                            if qr == 3:
                                kf = ptmp.tile([128, NQT], bf16, name="kf",
                                               tag="kf")
                                rope_evict(psk, kb_sb[:, 0:1], kf[:], ts)
                                for half in (0, 1):
                                    for dsth in (0, 1):
                                        nc.gpsimd.dma_start(
                                            out=ktd[half][dsth * 64:
                                                          (dsth + 1) * 64, ts],
                                            in_=kf[half * 64:(half + 1) * 64,
                                                   :])
                        return f

                    def vs(qr):
                        # V^T: kv dims on partitions, T on free — weights
                        # stationary, N=512 streams.
                        def f():
                            xq = x_tiles[tb]
                            if qr == 0:
                                state['v'] = pmain.tile([128, NQT], f32,
                                                        name="psv", tag="ps")
                            psv = state['v']
                            for c in range(qr * 4, qr * 4 + 4):
                                nc.tensor.matmul(psv[:], wv_sb[:, c, :],
                                                 xq[c // 4][:, c % 4, :],
                                                 start=(c == 0), stop=False)
                            if qr == 3:
                                nc.tensor.matmul(psv[:], vb_sb[:],
                                                 ones_row[:],
                                                 start=False, stop=True)
                                vt = ptmp.tile([128, NQT], bf16, name="vt",
                                               tag="vt")
                                nc.vector.tensor_copy(vt[:], psv[:])
                                state['vt'] = vt
                        return f

                    def vtr():
                        # PE-transpose V^T back to [keys, dims] for P@V.
                        vt = state['vt']
                        pstr = pmain.tile([128, NQT], bf16, name="pstr",
                                          tag="ps")
                        for ti in range(4):
                            nc.tensor.transpose(
                                pstr[:, ti * 128:(ti + 1) * 128],
                                vt[:, ti * 128:(ti + 1) * 128], ident[:])
                        for ti in range(4):
                            vi = tb * 4 + ti
                            nc.vector.memset(vsb[vi][:, 64:65], 1.0)
                            nc.vector.memset(vsb[vi][:, 129:130], 1.0)
                            nc.vector.tensor_copy(
                                vsb[vi][:, 0:64],
                                pstr[:, ti * 128:ti * 128 + 64])
                            nc.vector.tensor_copy(
                                vsb[vi][:, 65:129],
                                pstr[:, ti * 128 + 64:(ti + 1) * 128])

                    steps = [lambda qr=qr: load_x_quarter(tb, qr)
                             for qr in range(4)] if tb > 0 else []
                    for m in range(4):
                        steps += [qs(m, qr) for qr in range(4)]
                    steps += [ks(qr) for qr in range(4)]
                    steps += [vs(qr) for qr in range(4)]
                    steps.append(vtr)
                    return steps

                ag_tiles = {}

                def load_ag(j, m):
                    def f():
                        ag_sbm = agp.tile([128, NGROUPS, NQT], bf16,
                                          name="ag_sb")
                        nc.sync.dma_start(
                            out=ag_sbm[:],
                            in_=ag_out[j][m].rearrange("(g p) t -> p g t",
                                                       p=128))
                        ag_tiles[(j, m)] = ag_sbm
                    return f

                def oproj_evict(j, tt, pf):
                    ot = outp.tile([128, QDIM], bf16, name="ot", tag="ot")
                    nc.vector.tensor_copy(ot[:], pf[:])
                    nc.sync.dma_start(
                        out=out[j * NQT + tt * 128:
                                j * NQT + (tt + 1) * 128, :],
                        in_=ot[:])

                def oproj_steps(j):
                    state = {}

                    def os(tt, m):
                        def f():
                            if m == 0:
                                state[tt] = pmain.tile([128, QDIM], f32,
                                                       name="pf", tag="ps")
                            pf = state[tt]
                            agm = ag_tiles[(j, m)]
                            for g in range(NGROUPS):
                                nc.tensor.matmul(
                                    pf[:],
                                    agm[:, g, tt * 128:(tt + 1) * 128],
                                    ow_sb[:, g * 4 + m, :],
                                    start=(m == 0 and g == 0),
                                    stop=(m == 3 and g == NGROUPS - 1))
                            if m == 3:
                                oproj_evict(j, tt, pf)
                        return f

                    return [os(tt, m) for tt in range(4) for m in range(4)]

                def attn_hp(j, hp, filler):
                    qs_ = slice(j * NQT, (j + 1) * NQT)
                    kv = hp // 2
                    po0 = psO.tile([65, NQT], f32, name="po0", tag="po0")
                    po1 = psO.tile([65, NQT], f32, name="po1", tag="po1")
                    ch = plans[j]
                    n_ch = len(ch)
                    pss_t = {}

                    def scores(ci):
                        i = ch[ci][0]
                        pss = psS.tile([128, 1024], f32, name="pss",
                                       tag="pss")
                        # head-lo on array rows 0:64, head-hi on 64:128 —
                        # concurrent row-groups, separate PSUM banks
                        nc.tensor.matmul(
                            pss[:, 0:512],
                            ktd[kv][0:64, i * NKC:(i + 1) * NKC],
                            qt[hp][0:64, qs_], start=True, stop=True)
                        nc.tensor.matmul(
                            pss[:, 512:1024],
                            ktd[kv][64:128, i * NKC:(i + 1) * NKC],
                            qt[hp][64:128, qs_], start=True, stop=True)
                        pss_t[ci] = pss

                    scores(0)
                    for ci in range(n_ch):
                        if ci + 1 < n_ch:
                            scores(ci + 1)
                        i, kind, gi = ch[ci]
                        pss = pss_t.pop(ci)
                        pt = ptp.tile([128, 1024], bf16, name="pt", tag="pt")
                        nc.scalar.activation(pt[:], pss[:], AF.Exp,
                                             scale=0.125)
                        if kind == 1:
                            nc.vector.tensor_mul(pt[:, 0:512], pt[:, 0:512],
                                                 emt_sb[:, gi, :])
                            nc.vector.tensor_mul(pt[:, 512:1024],
                                                 pt[:, 512:1024],
                                                 emt_sb[:, gi, :])
                        # ~1us micro-steps of other PE work, emitted BETWEEN
                        # this chunk's scores and its P@V: the PE queue never
                        # head-of-line-stalls on the exp wait, which both
                        # fills the gap and keeps the HAM activity monitor
                        # seeing a busy PE (K=8/8, full clock).
                        filler.tick()
                        vsl = (vsb[i][:, 0:65] if kv == 0
                               else vsb[i][:, 65:130])
                        nc.tensor.matmul(po0[:], vsl, pt[:, 0:512],
                                         start=(ci == 0),
                                         stop=(ci == n_ch - 1))
                        nc.tensor.matmul(po1[:], vsl, pt[:, 512:1024],
                                         start=(ci == 0),
                                         stop=(ci == n_ch - 1))
                    at = evp.tile([128, NQT], bf16, name="at", tag="at",
                                  bufs=3)
                    for s, po in enumerate((po0, po1)):
                        # One fast copy frees the PSUM bank; the divide chain
                        # then runs off the PE critical path from SBUF.
                        pocp = evp.tile([65, NQT], f32, name="pocp",
                                        tag="pocp")
                        nc.vector.tensor_copy(pocp[:], po[:])
                        # reciprocal is ~6 cycles/elem serial per partition:
                        # spread the 512 sums over 128 lanes via two small
                        # DMAs so it costs ~0.2us instead of 1.7us
                        rs = evp.tile([128, 4], f32, name="rs", tag="rs")
                        nc.gpsimd.dma_start(out=rs[:], in_=pocp[64:65, :])
                        rr = evp.tile([128, 4], f32, name="rr", tag="rr")
                        nc.vector.reciprocal(rr[:], rs[:])
                        rc = evp.tile([1, NQT], f32, name="rc", tag="rc")
                        nc.gpsimd.dma_start(out=rc[:], in_=rr[:])
                        rb = evp.tile([64, NQT], f32, name="rb", tag="rb")
                        nc.gpsimd.partition_broadcast(rb[:], rc[:])
                        nc.vector.tensor_mul(at[s * 64:(s + 1) * 64, :],
                                             pocp[0:64, :], rb[:])
                    nc.sync.dma_start(out=ag_in[j][hp][:], in_=at[:])
                    nc.gpsimd.collective_compute(
                        "AllGather", ALU.bypass, replica_groups=rg,
                        ins=[ag_in[j][hp].opt()], outs=[ag_out[j][hp].opt()])

                def interleave(a, b, period=2):
                    """merge list b into list a, one b-item every `period`."""
                    out_, bi = [], 0
                    for k, u in enumerate(a):
                        out_.append(u)
                        if k % period == period - 1 and bi < len(b):
                            out_.append(b[bi])
                            bi += 1
                    out_ += b[bi:]
                    return out_

                class Pacer:
                    """Spread filler steps evenly over a window's chunk
                    slots so no stretch of attention runs bare (exp-bound
                    with an idling, HAM-cooling PE)."""

                    def __init__(self, steps, slots):
                        self.steps, self.slots = steps, max(slots, 1)
                        self.i, self.seen = 0, 0

                    def tick(self):
                        self.seen += 1
                        want = (len(self.steps) * self.seen) // self.slots
                        while self.i < min(want, len(self.steps)):
                            self.steps[self.i]()
                            self.i += 1

                    def drain(self):
                        while self.i < len(self.steps):
                            self.steps[self.i]()
                            self.i += 1

                # ---------------- the schedule ----------------
                for u in proj_steps(0):
                    u()
                for j in range(nj):
                    steps = []
                    if j == 0:
                        steps = proj_steps(1)
                    elif j == 1:
                        steps = proj_steps(2)
                    elif j == 2:
                        steps = interleave(proj_steps(3),
                                           [load_ag(0, m) for m in range(4)] +
                                           [load_ag(1, m) for m in range(4)])
                    else:
                        # window 3 is exp-heavy (~85us) with 64 chunk slots:
                        # all three earlier o_proj tiles fill it.  (2,m) ag
                        # buffers only free up as o_proj(0) retires (0,m).
                        steps = oproj_steps(0)
                        steps += [load_ag(2, m) for m in range(4)]
                        steps += oproj_steps(1)
                        steps.append(load_ag(3, 0))
                        steps += oproj_steps(2)
                        steps.append(load_ag(3, 1))
                    pacer = Pacer(steps, 4 * len(plans[j]))
                    for hp in range(4):
                        attn_hp(j, hp, pacer)
                    pacer.drain()
                # tail: last q-tile's o_proj accumulates m-major so only the
                # final quarter waits on the last (small) AllGather.
                jl = nj - 1
                load_ag(jl, 2)()
                load_ag(jl, 3)()
                pf = [pmain.tile([128, QDIM], f32, name="pf", tag="ps")
                      for _ in range(2)] + \
                     [psS.tile([128, QDIM], f32, name="pf2", tag="pss")
                      for _ in range(2)]
                for m in range(4):
                    agm = ag_tiles[(jl, m)]
                    for tt in range(4):
                        for g in range(NGROUPS):
                            nc.tensor.matmul(
                                pf[tt][:],
                                agm[:, g, tt * 128:(tt + 1) * 128],
                                ow_sb[:, g * 4 + m, :],
                                start=(m == 0 and g == 0),
                                stop=(m == 3 and g == NGROUPS - 1))
                for tt in range(4):
                    oproj_evict(jl, tt, pf[tt])

    nc.compile()
    return nc


def prep_inputs(hidden, positions, mask, q_w, q_b, k_w, k_b, v_w, v_b, o_w,
                emt_tiles):
    """Host-side shard + transform -> in_maps for the 8 cores."""
    B, T, _ = hidden.shape
    nhc = HID // 128
    pos = np.asarray(positions)[0].astype(np.float32)
    inv_freq = (1.0 / (THETA ** (np.arange(0, D, 2, dtype=np.float32) / D)))
    freqs = pos[:, None] * inv_freq[None, :]          # (T, 32)
    cos_t, sin_t = np.cos(freqs).T, np.sin(freqs).T   # (32, T)
    cos_tab = np.ascontiguousarray(np.tile(cos_t, (4, 1)), dtype=np.float32)
    ssin_tab = np.ascontiguousarray(
        np.concatenate([sin_t, -sin_t, sin_t, -sin_t], axis=0),
        dtype=np.float32)

    if emt_tiles:
        emt_arr = np.ascontiguousarray(
            np.stack(emt_tiles, axis=1)).astype(BF16)  # [128, n_emt, 512]
    else:
        emt_arr = np.zeros((128, 1, NQT), BF16)

    def chunked(w):   # [HID, O] -> [128, nhc, O]
        return np.ascontiguousarray(
            w.reshape(nhc, 128, w.shape[1]).transpose(1, 0, 2)).astype(BF16)

    nj = T // NQT
    xts = []
    for b in range(B):
        a = np.asarray(hidden[b], np.float32)          # [T, HID]
        a = a.reshape(nj, NQT, nhc, 128)               # [tb, u, c, p]
        xts.append(np.ascontiguousarray(
            a.transpose(0, 3, 2, 1)).astype(BF16))     # [tb, p, c, u]

    in_maps = []
    for c in range(NCORES):
        b, g = c // NGROUPS, c % NGROUPS
        qsl = slice(QDIM * g, QDIM * (g + 1))
        ksl = slice(KVDIM * g, KVDIM * (g + 1))
        in_maps.append({
            "xt": xts[b],
            "wqt": chunked(np.asarray(q_w[qsl, :]).T),
            "wkt": chunked(np.asarray(k_w[ksl, :]).T),
            "wvt": chunked(np.asarray(v_w[ksl, :]).T),
            "qb": np.ascontiguousarray(
                np.asarray(q_b[qsl], np.float32).reshape(4, 128).T),
            "kb": np.ascontiguousarray(
                np.asarray(k_b[ksl], np.float32).reshape(1, 128).T),
            "vb": np.asarray(v_b[ksl]).astype(BF16).reshape(1, KVDIM),
            "cosq": cos_tab,
            "ssin": ssin_tab,
            "emt": emt_arr,
            "owt": chunked(np.asarray(o_w).T[:, qsl]),
        })
    return in_maps


def _ensure_ntff_hook():
    """Provide antenv.axon_hooks in containers whose antenv stub lacks it,
    wiring the ctypes NTFF profiler from the injected axon boot package."""
    import sys
    import types
    try:
        from antenv.axon_hooks import get_axon_ntff_profile_hook  # noqa: F401
        return True
    except ImportError:
        pass
    try:
        import antenv
        from trn_agent_boot.trn_boot import _ntff_profile_via_ctypes
        hook = _ntff_profile_via_ctypes("/opt/axon/libaxon_pjrt.so")
        if hook is None:
            return False
        mod = types.ModuleType("antenv.axon_hooks")
        state = {"h": hook}
        mod.get_axon_ntff_profile_hook = lambda: state["h"]
        mod.set_axon_ntff_profile_hook = lambda h: state.__setitem__("h", h)
        sys.modules["antenv.axon_hooks"] = mod
        antenv.axon_hooks = mod
        return True
    except Exception:
        return False


def kernel(hidden, positions, mask, q_w, q_b, k_w, k_b, v_w, v_b, o_w):
    global LAST_RESULT
    from concourse import bass_utils

    hidden = np.asarray(hidden)
    B, T, _ = hidden.shape
    mask_key = (T, hash(np.asarray(mask).tobytes()))
    if mask_key not in _cache:
        plans, emt_tiles = plan_mask(mask, T)
        nc = build_graph(T, plans, len(emt_tiles))
        _cache[mask_key] = (nc, emt_tiles)
    nc, emt_tiles = _cache[mask_key]

    in_maps = prep_inputs(hidden, positions, mask, q_w, q_b, k_w, k_b,
                          v_w, v_b, o_w, emt_tiles)
    trace = os.environ.get("BASS_KERNEL_TRACE", "0") == "1"
    if trace:
        trace = _ensure_ntff_hook()
    res = bass_utils.run_bass_kernel_spmd(nc, in_maps,
                                          core_ids=list(range(NCORES)),
                                          trace=trace)
    LAST_RESULT = res
    out = np.zeros((B, T, HID), np.float32)
    for c in range(NCORES):
        b, g = c // NGROUPS, c % NGROUPS
        out[b, :, QDIM * g:QDIM * (g + 1)] = \
            res.results[c]["out"].astype(np.float32)
    return out


# revision 15
# speedup vs baseline: 1.0010x; 1.0010x over previous
"""GQA attention (32 q heads / 8 kv heads, D=64, HID=2048, B=2, T=2048)
distributed over 8 TRN2 NeuronCores.

Sharding: 2-way data parallel (batch) x 4-way tensor parallel (head groups).
Core c handles batch c//4 and head group g=c%4 (q heads [8g,8g+8), kv heads
[2g,2g+2)).  Each core projects Q^T/K^T (transposed layout: head-dims on
partitions, T on free axis), computes V^T the same cheap way (weights
stationary, N=512 streams) and PE-transposes it into the [keys, dims] layout
that P@V needs.  Scores^T = K @ Q^T per head with keys on partitions, exp via
ScalarE (no max-subtraction needed at these magnitudes; masked entries
multiply to exactly 0 by a host-precomputed exp(mask) factor), then
out^T = Vext^T @ P^T where Vext carries a ones column producing the softmax
denominators for free.

The attention phase is ScalarE(exp)-bound (~1.33us per key-chunk vs ~0.78us
of matmul), so all other PE work — next block's projections, o_proj of
previous q-tiles, gather loads — is chopped into ~1us micro-steps and
interleaved ONE PER KEY-CHUNK into the attention emission, with the score
matmuls software-pipelined one chunk ahead so the exp stream never waits.
Attention outputs are AllGathered per head-pair (16 gathers of 128KB); a
dummy gather at t=0 absorbs the collective warm-up, and the last q-tile's
o_proj accumulates m-major so only its final quarter waits on the last
gather.  All host-side layouts give every DMA >=4KB contiguous per partition
(hardware DGE fast path).
"""

import os
import numpy as np
import ml_dtypes

BF16 = ml_dtypes.bfloat16

HQ, HKV, D, HID, THETA = 32, 8, 64, 2048, 10000.0
NCORES, NGROUPS = 8, 4
QDIM = HQ * D // NGROUPS        # 512 q dims per core
KVDIM = HKV * D // NGROUPS      # 128 kv dims per core
NQT = 512                       # query tile (free dim per PSUM bank)
NKC = 128                       # key chunk (partition dim)

_cache = {}
LAST_RESULT = None              # BassKernelResults of the most recent run


def plan_mask(mask, T):
    """Classify (key-chunk i, q-tile j) tiles of exp(mask).T.

    Returns (plans, emt_tiles): plans[j] = list of (i, kind, emt_idx) where
    kind 0 = no mask needed (exp(mask)==1 on tile), kind 1 = multiply by
    emt_tiles[emt_idx].  All-zero tiles are skipped entirely (they contribute
    nothing to P@V nor to the softmax denominator).
    """
    m = np.asarray(mask, dtype=np.float32).reshape(T, T)
    with np.errstate(under="ignore"):
        em = np.exp(m).T.astype(np.float32)   # em[k, q] = exp(mask[q, k])
    nj, nk = T // NQT, T // NKC
    plans, emt_tiles = [], []
    for j in range(nj):
        pj = []
        for i in range(nk):
            t = em[i * NKC:(i + 1) * NKC, j * NQT:(j + 1) * NQT]
            if not t.any():
                continue
            if (t == 1.0).all():
                pj.append((i, 0, -1))
            else:
                pj.append((i, 1, len(emt_tiles)))
                emt_tiles.append(t.astype(BF16))
        plans.append(pj)
    return plans, emt_tiles


def build_graph(T, plans, n_emt):
    """Build the SPMD Bacc graph (same on all 8 cores; shards arrive as data)."""
    import concourse.bass as bass  # noqa: F401
    import concourse.mybir as mybir
    import concourse.tile as tile
    from concourse import bacc, masks

    f32, bf16 = mybir.dt.float32, mybir.dt.bfloat16
    AF, ALU = mybir.ActivationFunctionType, mybir.AluOpType

    nj = T // NQT          # q tiles
    nhc = HID // 128       # contraction chunks over hidden dim (16)
    noc = (HQ * D) // 128  # contraction chunks over gathered head dim (16)
    nem = max(n_emt, 1)
    assert nj == 4, "schedule below is specialized for T=2048"

    nc = bacc.Bacc("TRN2", target_bir_lowering=False, debug=False,
                   num_devices=NCORES)

    # host-prepped layouts: every tensor reads contiguous >=4KB per partition
    xt = nc.dram_tensor("xt", [nj, 128, nhc, NQT], bf16,
                        kind="ExternalInput").ap()
    wqt = nc.dram_tensor("wqt", [128, nhc, QDIM], bf16,
                         kind="ExternalInput").ap()
    wkt = nc.dram_tensor("wkt", [128, nhc, KVDIM], bf16,
                         kind="ExternalInput").ap()
    wvt = nc.dram_tensor("wvt", [128, nhc, KVDIM], bf16,
                         kind="ExternalInput").ap()
    qb = nc.dram_tensor("qb", [128, 4], f32, kind="ExternalInput").ap()
    kb = nc.dram_tensor("kb", [128, 1], f32, kind="ExternalInput").ap()
    vb = nc.dram_tensor("vb", [1, KVDIM], bf16, kind="ExternalInput").ap()
    cosq = nc.dram_tensor("cosq", [128, T], f32, kind="ExternalInput").ap()
    ssin = nc.dram_tensor("ssin", [128, T], f32, kind="ExternalInput").ap()
    emt = nc.dram_tensor("emt", [128, nem, NQT], bf16,
                         kind="ExternalInput").ap()
    owt = nc.dram_tensor("owt", [128, noc, QDIM], bf16,
                         kind="ExternalInput").ap()
    out = nc.dram_tensor("out", [T, QDIM], bf16, kind="ExternalOutput").ap()

    rg = [[0, 1, 2, 3], [4, 5, 6, 7]]

    with tile.TileContext(nc) as tc:
        with tc.tile_pool(name="dramp", bufs=1, space="DRAM") as dramp:
            ag_in = [[dramp.tile([128, NQT], bf16, name=f"agin{j}_{m}")
                      for m in range(4)] for j in range(nj)]
            ag_out = [[dramp.tile([NGROUPS * 128, NQT], bf16,
                                  name=f"agout{j}_{m}")
                       for m in range(4)] for j in range(nj)]
            dum_in = dramp.tile([1, 128], bf16, name="dum_in")
            dum_out = dramp.tile([4, 128], bf16, name="dum_out")

        with tc.tile_pool(name="persist", bufs=1) as pp:
            # Q^T per head-pair chunk: [128 (2 heads x 64), T]
            qt = [pp.tile([128, T], bf16, name=f"qt{m}") for m in range(4)]
            # K^T duplicated per kv head: [128 = kv dup'd twice, T]
            ktd = [pp.tile([128, T], bf16, name=f"ktd{k}") for k in range(2)]
            # V per key chunk: [128 keys, 130] (V0|one|V1|one)
            vsb = [pp.tile([128, 130], bf16, name=f"v{i}")
                   for i in range(T // NKC)]
            ow_sb = pp.tile([128, noc, QDIM], bf16, name="ow_sb")
            wqq = [pp.tile([128, 4, QDIM], bf16, name=f"wqq{r}")
                   for r in range(4)]
            wk_sb = pp.tile([128, nhc, KVDIM], bf16, name="wk_sb")
            wv_sb = pp.tile([128, nhc, KVDIM], bf16, name="wv_sb")
            cos_sb = pp.tile([128, T], f32, name="cos_sb")
            ssin_sb = pp.tile([128, T], f32, name="ssin_sb")
            emt_sb = pp.tile([128, nem, NQT], bf16, name="emt_sb")
            ident = pp.tile([128, 128], bf16, name="ident")
            vb_sb = pp.tile([1, KVDIM], bf16, name="vb_sb")
            ones_row = pp.tile([1, NQT], bf16, name="ones_row")
            qb_sb = pp.tile([128, 4], f32, name="qb_sb")
            kb_sb = pp.tile([128, 1], f32, name="kb_sb")

            nc.sync.dma_start(out=qb_sb[:], in_=qb)
            nc.sync.dma_start(out=kb_sb[:], in_=kb)
            nc.sync.dma_start(out=vb_sb[:], in_=vb)
            nc.vector.memset(ones_row[:], 1.0)
            # touch Exp once so the ACT table load (~2.7us) happens during
            # the startup DMA wait, not before the first real softmax
            warm = pp.tile([1, 2], bf16, name="warm")
            nc.scalar.activation(warm[:], ones_row[0:1, 0:2], AF.Exp,
                                 scale=0.125)
            # dummy gather: absorbs the one-time collective barrier/warm-up
            # (~12us trigger delay + ~2.5x duration) before the first real one
            nc.sync.dma_start(out=dum_in[:], in_=ones_row[0:1, 0:128])
            nc.gpsimd.collective_compute(
                "AllGather", ALU.bypass, replica_groups=rg,
                ins=[dum_in.opt()], outs=[dum_out.opt()])
            masks.make_identity(nc, ident[:])

            with tc.tile_pool(name="projx", bufs=2) as px, \
                 tc.tile_pool(name="projtmp", bufs=2) as ptmp, \
                 tc.tile_pool(name="pmain", bufs=2, space="PSUM") as pmain, \
                 tc.tile_pool(name="psS", bufs=2, space="PSUM") as psS, \
                 tc.tile_pool(name="psO", bufs=1, space="PSUM") as psO, \
                 tc.tile_pool(name="ptp", bufs=3) as ptp, \
                 tc.tile_pool(name="evp", bufs=2) as evp, \
                 tc.tile_pool(name="agp", bufs=8) as agp, \
                 tc.tile_pool(name="outp", bufs=2) as outp:

                x_tiles = {}

                def load_x_quarter(tb, qr):
                    xq = px.tile([128, 4, NQT], bf16, name="x_sb", tag="x_sb",
                                 bufs=8)
                    nc.sync.dma_start(out=xq[:],
                                      in_=xt[tb, :, qr * 4:(qr + 1) * 4, :])
                    x_tiles.setdefault(tb, []).append(xq)

                # startup: x(0) quarters interleaved with wq quarters so the
                # first Q matmuls start after ~1MB instead of ~5MB of DMA.
                for qr in range(4):
                    load_x_quarter(0, qr)
                    nc.sync.dma_start(out=wqq[qr][:],
                                      in_=wqt[:, qr * 4:(qr + 1) * 4, :])
                nc.sync.dma_start(out=wk_sb[:], in_=wkt)
                nc.sync.dma_start(out=wv_sb[:], in_=wvt)
                nc.sync.dma_start(out=cos_sb[:], in_=cosq)
                nc.sync.dma_start(out=ssin_sb[:], in_=ssin)
                nc.sync.dma_start(out=emt_sb[:], in_=emt)
                nc.sync.dma_start(out=ow_sb[:], in_=owt)

                def rope_evict(ps, bias_col, dst, ts):
                    """dst = RoPE(ps + bias) cast to bf16."""
                    t2 = ptmp.tile([128, NQT], f32, name="t2", tag="t2")
                    nc.vector.scalar_tensor_tensor(
                        t2[:], ps[:], bias_col, ssin_sb[:, ts],
                        op0=ALU.add, op1=ALU.mult)
                    t2s = ptmp.tile([128, NQT], f32, name="t2s", tag="t2s")
                    for blk in range(4):
                        sb = blk ^ 1
                        # off the sync queue: keeps it free for attention
                        nc.gpsimd.dma_start(
                            out=t2s[blk * 32:(blk + 1) * 32, :],
                            in_=t2[sb * 32:(sb + 1) * 32, :])
                    t1 = ptmp.tile([128, NQT], f32, name="t1", tag="t1")
                    nc.vector.scalar_tensor_tensor(
                        t1[:], ps[:], bias_col, cos_sb[:, ts],
                        op0=ALU.add, op1=ALU.mult)
                    nc.vector.tensor_add(dst, t1[:], t2s[:])

                def proj_steps(tb):
                    """Projection of T-block tb as ~1us micro-steps."""
                    ts = slice(tb * NQT, (tb + 1) * NQT)
                    state = {}

                    def qs(m, qr):
                        def f():
                            xq = x_tiles[tb]
                            if qr == 0:
                                state[m] = pmain.tile([128, NQT], f32,
                                                      name="ps", tag="ps")
                            ps = state[m]
                            for c in range(qr * 4, qr * 4 + 4):
                                nc.tensor.matmul(
                                    ps[:],
                                    wqq[c // 4][:, c % 4,
                                                m * 128:(m + 1) * 128],
                                    xq[c // 4][:, c % 4, :],
                                    start=(c == 0), stop=(c == nhc - 1))
                            if qr == 3:
                                rope_evict(ps, qb_sb[:, m:m + 1],
                                           qt[m][:, ts], ts)
                        return f

                    def ks(qr):
                        def f():
                            xq = x_tiles[tb]
                            if qr == 0:
                                state['k'] = pmain.tile([128, NQT], f32,
                                                        name="psk", tag="ps")
                            psk = state['k']
                            for c in range(qr * 4, qr * 4 + 4):
                                nc.tensor.matmul(psk[:], wk_sb[:, c, :],
                                                 xq[c // 4][:, c % 4, :],
                                                 start=(c == 0),
                                                 stop=(c == nhc - 1))
                            if qr == 3:
                                kf = ptmp.tile([128, NQT], bf16, name="kf",
                                               tag="kf")
                                rope_evict(psk, kb_sb[:, 0:1], kf[:], ts)
                                for half in (0, 1):
                                    for dsth in (0, 1):
                                        nc.gpsimd.dma_start(
                                            out=ktd[half][dsth * 64:
                                                          (dsth + 1) * 64, ts],
                                            in_=kf[half * 64:(half + 1) * 64,
                                                   :])
                        return f

                    def vs(qr):
                        # V^T: kv dims on partitions, T on free — weights
                        # stationary, N=512 streams.
                        def f():
                            xq = x_tiles[tb]
                            if qr == 0:
                                state['v'] = pmain.tile([128, NQT], f32,
                                                        name="psv", tag="ps")
                            psv = state['v']
                            for c in range(qr * 4, qr * 4 + 4):
                                nc.tensor.matmul(psv[:], wv_sb[:, c, :],
                                                 xq[c // 4][:, c % 4, :],
                                                 start=(c == 0), stop=False)
                            if qr == 3:
                                nc.tensor.matmul(psv[:], vb_sb[:],
                                                 ones_row[:],
                                                 start=False, stop=True)
                                vt = ptmp.tile([128, NQT], bf16, name="vt",
                                               tag="vt")
                                nc.vector.tensor_copy(vt[:], psv[:])
                                state['vt'] = vt
                        return f

                    def vtr():
                        # PE-transpose V^T back to [keys, dims] for P@V.
                        vt = state['vt']
                        pstr = pmain.tile([128, NQT], bf16, name="pstr",
                                          tag="ps")
                        for ti in range(4):
                            nc.tensor.transpose(
                                pstr[:, ti * 128:(ti + 1) * 128],
                                vt[:, ti * 128:(ti + 1) * 128], ident[:])
                        for ti in range(4):
                            vi = tb * 4 + ti
                            nc.vector.memset(vsb[vi][:, 64:65], 1.0)
                            nc.vector.memset(vsb[vi][:, 129:130], 1.0)
                            nc.vector.tensor_copy(
                                vsb[vi][:, 0:64],
                                pstr[:, ti * 128:ti * 128 + 64])
                            nc.vector.tensor_copy(
                                vsb[vi][:, 65:129],
                                pstr[:, ti * 128 + 64:(ti + 1) * 128])

                    steps = [lambda qr=qr: load_x_quarter(tb, qr)
                             for qr in range(4)] if tb > 0 else []
                    for m in range(4):
                        steps += [qs(m, qr) for qr in range(4)]
                    steps += [ks(qr) for qr in range(4)]
                    steps += [vs(qr) for qr in range(4)]
                    steps.append(vtr)
                    return steps

                ag_tiles = {}

                def load_ag(j, m):
                    def f():
                        ag_sbm = agp.tile([128, NGROUPS, NQT], bf16,
                                          name="ag_sb")
                        nc.sync.dma_start(
                            out=ag_sbm[:],
                            in_=ag_out[j][m].rearrange("(g p) t -> p g t",
                                                       p=128))
                        ag_tiles[(j, m)] = ag_sbm
                    return f

                def oproj_evict(j, tt, pf):
                    ot = outp.tile([128, QDIM], bf16, name="ot", tag="ot")
                    nc.vector.tensor_copy(ot[:], pf[:])
                    nc.sync.dma_start(
                        out=out[j * NQT + tt * 128:
                                j * NQT + (tt + 1) * 128, :],
                        in_=ot[:])

                def oproj_steps(j):
                    state = {}

                    def os(tt, m):
                        def f():
                            if m == 0:
                                state[tt] = pmain.tile([128, QDIM], f32,
                                                       name="pf", tag="ps")
                            pf = state[tt]
                            agm = ag_tiles[(j, m)]
                            for g in range(NGROUPS):
                                nc.tensor.matmul(
                                    pf[:],
                                    agm[:, g, tt * 128:(tt + 1) * 128],
                                    ow_sb[:, g * 4 + m, :],
                                    start=(m == 0 and g == 0),
                                    stop=(m == 3 and g == NGROUPS - 1))
                            if m == 3:
                                oproj_evict(j, tt, pf)
                        return f

                    return [os(tt, m) for tt in range(4) for m in range(4)]

                def attn_hp(j, hp, filler):
                    qs_ = slice(j * NQT, (j + 1) * NQT)
                    kv = hp // 2
                    po0 = psO.tile([65, NQT], f32, name="po0", tag="po0")
                    po1 = psO.tile([65, NQT], f32, name="po1", tag="po1")
                    ch = plans[j]
                    n_ch = len(ch)
                    pss_t = {}

                    def scores(ci):
                        i = ch[ci][0]
                        pss = psS.tile([128, 1024], f32, name="pss",
                                       tag="pss")
                        # head-lo on array rows 0:64, head-hi on 64:128 —
                        # concurrent row-groups, separate PSUM banks
                        nc.tensor.matmul(
                            pss[:, 0:512],
                            ktd[kv][0:64, i * NKC:(i + 1) * NKC],
                            qt[hp][0:64, qs_], start=True, stop=True)
                        nc.tensor.matmul(
                            pss[:, 512:1024],
                            ktd[kv][64:128, i * NKC:(i + 1) * NKC],
                            qt[hp][64:128, qs_], start=True, stop=True)
                        pss_t[ci] = pss

                    scores(0)
                    for ci in range(n_ch):
                        if ci + 1 < n_ch:
                            scores(ci + 1)
                        i, kind, gi = ch[ci]
                        pss = pss_t.pop(ci)
                        pt = ptp.tile([128, 1024], bf16, name="pt", tag="pt")
                        nc.scalar.activation(pt[:], pss[:], AF.Exp,
                                             scale=0.125)
                        if kind == 1:
                            nc.vector.tensor_mul(pt[:, 0:512], pt[:, 0:512],
                                                 emt_sb[:, gi, :])
                            nc.vector.tensor_mul(pt[:, 512:1024],
                                                 pt[:, 512:1024],
                                                 emt_sb[:, gi, :])
                        # ~1us micro-steps of other PE work, emitted BETWEEN
                        # this chunk's scores and its P@V: the PE queue never
                        # head-of-line-stalls on the exp wait, which both
                        # fills the gap and keeps the HAM activity monitor
                        # seeing a busy PE (K=8/8, full clock).  Double pull
                        # on the first chunk: covers the previous head-pair's
                        # softmax-evict chain (the new po accumulators wait
                        # on its PSUM copies).
                        filler.tick()
                        if ci == 0:
                            filler.tick()
                        vsl = (vsb[i][:, 0:65] if kv == 0
                               else vsb[i][:, 65:130])
                        nc.tensor.matmul(po0[:], vsl, pt[:, 0:512],
                                         start=(ci == 0),
                                         stop=(ci == n_ch - 1))
                        nc.tensor.matmul(po1[:], vsl, pt[:, 512:1024],
                                         start=(ci == 0),
                                         stop=(ci == n_ch - 1))
                    at = evp.tile([128, NQT], bf16, name="at", tag="at",
                                  bufs=3)
                    for s, po in enumerate((po0, po1)):
                        # One fast copy frees the PSUM bank; the divide chain
                        # then runs off the PE critical path from SBUF.
                        pocp = evp.tile([65, NQT], f32, name="pocp",
                                        tag="pocp")
                        nc.vector.tensor_copy(pocp[:], po[:])
                        # reciprocal is ~6 cycles/elem serial per partition:
                        # spread the 512 sums over 128 lanes via two small
                        # DMAs so it costs ~0.2us instead of 1.7us
                        rs = evp.tile([128, 4], f32, name="rs", tag="rs")
                        nc.gpsimd.dma_start(out=rs[:], in_=pocp[64:65, :])
                        rr = evp.tile([128, 4], f32, name="rr", tag="rr")
                        nc.vector.reciprocal(rr[:], rs[:])
                        rc = evp.tile([1, NQT], f32, name="rc", tag="rc")
                        nc.gpsimd.dma_start(out=rc[:], in_=rr[:])
                        rb = evp.tile([64, NQT], f32, name="rb", tag="rb")
                        nc.gpsimd.partition_broadcast(rb[:], rc[:])
                        nc.vector.tensor_mul(at[s * 64:(s + 1) * 64, :],
                                             pocp[0:64, :], rb[:])
                    nc.sync.dma_start(out=ag_in[j][hp][:], in_=at[:])
                    nc.gpsimd.collective_compute(
                        "AllGather", ALU.bypass, replica_groups=rg,
                        ins=[ag_in[j][hp].opt()], outs=[ag_out[j][hp].opt()])

                def interleave(a, b, period=2):
                    """merge list b into list a, one b-item every `period`."""
                    out_, bi = [], 0
                    for k, u in enumerate(a):
                        out_.append(u)
                        if k % period == period - 1 and bi < len(b):
                            out_.append(b[bi])
                            bi += 1
                    out_ += b[bi:]
                    return out_

                class Pacer:
                    """Spread filler steps evenly over a window's chunk
                    slots so no stretch of attention runs bare (exp-bound
                    with an idling, HAM-cooling PE)."""

                    def __init__(self, steps, slots):
                        self.steps, self.slots = steps, max(slots, 1)
                        self.i, self.seen = 0, 0

                    def tick(self):
                        self.seen += 1
                        want = (len(self.steps) * self.seen) // self.slots
                        while self.i < min(want, len(self.steps)):
                            self.steps[self.i]()
                            self.i += 1

                    def drain(self):
                        while self.i < len(self.steps):
                            self.steps[self.i]()
                            self.i += 1

                # ---------------- the schedule ----------------
                for u in proj_steps(0):
                    u()
                for j in range(nj):
                    steps = []
                    if j == 0:
                        steps = proj_steps(1)
                    elif j == 1:
                        steps = proj_steps(2)
                    elif j == 2:
                        steps = interleave(proj_steps(3),
                                           [load_ag(0, m) for m in range(4)] +
                                           [load_ag(1, m) for m in range(4)])
                    else:
                        # window 3 is exp-heavy (~85us) with 64 chunk slots:
                        # all three earlier o_proj tiles fill it.  (2,m) ag
                        # buffers only free up as o_proj(0) retires (0,m).
                        steps = oproj_steps(0)
                        steps += [load_ag(2, m) for m in range(4)]
                        steps += oproj_steps(1)
                        steps += oproj_steps(2)
                    pacer = Pacer(steps, 4 * len(plans[j]))
                    for hp in range(4):
                        attn_hp(j, hp, pacer)
                    pacer.drain()
                # tail: last q-tile's o_proj accumulates m-major so only the
                # final quarter waits on the last (small) AllGather.  ALL of
                # the last tile's gather loads go here — a load emitted
                # mid-window waits on its gather ON THE SYNC QUEUE and
                # head-of-line-blocks the at->gather-input DMAs behind it,
                # cascading the remaining gathers ~25us late.
                jl = nj - 1
                for m in range(4):
                    load_ag(jl, m)()
                pf = [pmain.tile([128, QDIM], f32, name="pf", tag="ps")
                      for _ in range(2)] + \
                     [psS.tile([128, QDIM], f32, name="pf2", tag="pss")
                      for _ in range(2)]
                for m in range(4):
                    agm = ag_tiles[(jl, m)]
                    for tt in range(4):
                        for g in range(NGROUPS):
                            nc.tensor.matmul(
                                pf[tt][:],
                                agm[:, g, tt * 128:(tt + 1) * 128],
                                ow_sb[:, g * 4 + m, :],
                                start=(m == 0 and g == 0),
                                stop=(m == 3 and g == NGROUPS - 1))
                for tt in range(4):
                    oproj_evict(jl, tt, pf[tt])

    nc.compile()
    return nc


def prep_inputs(hidden, positions, mask, q_w, q_b, k_w, k_b, v_w, v_b, o_w,
                emt_tiles):
    """Host-side shard + transform -> in_maps for the 8 cores."""
    B, T, _ = hidden.shape
    nhc = HID // 128
    pos = np.asarray(positions)[0].astype(np.float32)
    inv_freq = (1.0 / (THETA ** (np.arange(0, D, 2, dtype=np.float32) / D)))
    freqs = pos[:, None] * inv_freq[None, :]          # (T, 32)
    cos_t, sin_t = np.cos(freqs).T, np.sin(freqs).T   # (32, T)
    cos_tab = np.ascontiguousarray(np.tile(cos_t, (4, 1)), dtype=np.float32)
    ssin_tab = np.ascontiguousarray(
        np.concatenate([sin_t, -sin_t, sin_t, -sin_t], axis=0),
        dtype=np.float32)

    if emt_tiles:
        emt_arr = np.ascontiguousarray(
            np.stack(emt_tiles, axis=1)).astype(BF16)  # [128, n_emt, 512]
    else:
        emt_arr = np.zeros((128, 1, NQT), BF16)

    def chunked(w):   # [HID, O] -> [128, nhc, O]
        return np.ascontiguousarray(
            w.reshape(nhc, 128, w.shape[1]).transpose(1, 0, 2)).astype(BF16)

    nj = T // NQT
    xts = []
    for b in range(B):
        a = np.asarray(hidden[b], np.float32)          # [T, HID]
        a = a.reshape(nj, NQT, nhc, 128)               # [tb, u, c, p]
        xts.append(np.ascontiguousarray(
            a.transpose(0, 3, 2, 1)).astype(BF16))     # [tb, p, c, u]

    in_maps = []
    for c in range(NCORES):
        b, g = c // NGROUPS, c % NGROUPS
        qsl = slice(QDIM * g, QDIM * (g + 1))
        ksl = slice(KVDIM * g, KVDIM * (g + 1))
        in_maps.append({
            "xt": xts[b],
            "wqt": chunked(np.asarray(q_w[qsl, :]).T),
            "wkt": chunked(np.asarray(k_w[ksl, :]).T),
            "wvt": chunked(np.asarray(v_w[ksl, :]).T),
            "qb": np.ascontiguousarray(
                np.asarray(q_b[qsl], np.float32).reshape(4, 128).T),
            "kb": np.ascontiguousarray(
                np.asarray(k_b[ksl], np.float32).reshape(1, 128).T),
            "vb": np.asarray(v_b[ksl]).astype(BF16).reshape(1, KVDIM),
            "cosq": cos_tab,
            "ssin": ssin_tab,
            "emt": emt_arr,
            "owt": chunked(np.asarray(o_w).T[:, qsl]),
        })
    return in_maps


def _ensure_ntff_hook():
    """Provide antenv.axon_hooks in containers whose antenv stub lacks it,
    wiring the ctypes NTFF profiler from the injected axon boot package."""
    import sys
    import types
    try:
        from antenv.axon_hooks import get_axon_ntff_profile_hook  # noqa: F401
        return True
    except ImportError:
        pass
    try:
        import antenv
        from trn_agent_boot.trn_boot import _ntff_profile_via_ctypes
        hook = _ntff_profile_via_ctypes("/opt/axon/libaxon_pjrt.so")
        if hook is None:
            return False
        mod = types.ModuleType("antenv.axon_hooks")
        state = {"h": hook}
        mod.get_axon_ntff_profile_hook = lambda: state["h"]
        mod.set_axon_ntff_profile_hook = lambda h: state.__setitem__("h", h)
        sys.modules["antenv.axon_hooks"] = mod
        antenv.axon_hooks = mod
        return True
    except Exception:
        return False


def kernel(hidden, positions, mask, q_w, q_b, k_w, k_b, v_w, v_b, o_w):
    global LAST_RESULT
    from concourse import bass_utils

    hidden = np.asarray(hidden)
    B, T, _ = hidden.shape
    mask_key = (T, hash(np.asarray(mask).tobytes()))
    if mask_key not in _cache:
        plans, emt_tiles = plan_mask(mask, T)
        nc = build_graph(T, plans, len(emt_tiles))
        _cache[mask_key] = (nc, emt_tiles)
    nc, emt_tiles = _cache[mask_key]

    in_maps = prep_inputs(hidden, positions, mask, q_w, q_b, k_w, k_b,
                          v_w, v_b, o_w, emt_tiles)
    trace = os.environ.get("BASS_KERNEL_TRACE", "0") == "1"
    if trace:
        trace = _ensure_ntff_hook()
    res = bass_utils.run_bass_kernel_spmd(nc, in_maps,
                                          core_ids=list(range(NCORES)),
                                          trace=trace)
    LAST_RESULT = res
    out = np.zeros((B, T, HID), np.float32)
    for c in range(NCORES):
        b, g = c // NGROUPS, c % NGROUPS
        out[b, :, QDIM * g:QDIM * (g + 1)] = \
            res.results[c]["out"].astype(np.float32)
    return out


# revision 17
# speedup vs baseline: 1.0060x; 1.0050x over previous
"""GQA attention (32 q heads / 8 kv heads, D=64, HID=2048, B=2, T=2048)
distributed over 8 TRN2 NeuronCores.

Sharding: 2-way data parallel (batch) x 4-way tensor parallel (head groups).
Core c handles batch c//4 and head group g=c%4 (q heads [8g,8g+8), kv heads
[2g,2g+2)).  Each core projects Q^T/K^T (transposed layout: head-dims on
partitions, T on free axis), computes V^T the same cheap way (weights
stationary, N=512 streams) and PE-transposes it into the [keys, dims] layout
that P@V needs.  Scores^T = K @ Q^T per head with keys on partitions, exp via
ScalarE (no max-subtraction needed at these magnitudes; masked entries
multiply to exactly 0 by a host-precomputed exp(mask) factor), then
out^T = Vext^T @ P^T where Vext carries a ones column producing the softmax
denominators for free.

The attention phase is ScalarE(exp)-bound (~1.33us per key-chunk vs ~0.78us
of matmul), so all other PE work — next block's projections, o_proj of
previous q-tiles, gather loads — is chopped into ~1us micro-steps and
interleaved ONE PER KEY-CHUNK into the attention emission, with the score
matmuls software-pipelined one chunk ahead so the exp stream never waits.
Attention outputs are AllGathered per head-pair (16 gathers of 128KB); a
dummy gather at t=0 absorbs the collective warm-up, and the last q-tile's
o_proj accumulates m-major so only its final quarter waits on the last
gather.  All host-side layouts give every DMA >=4KB contiguous per partition
(hardware DGE fast path).
"""

import os
import numpy as np
import ml_dtypes

BF16 = ml_dtypes.bfloat16

HQ, HKV, D, HID, THETA = 32, 8, 64, 2048, 10000.0
NCORES, NGROUPS = 8, 4
QDIM = HQ * D // NGROUPS        # 512 q dims per core
KVDIM = HKV * D // NGROUPS      # 128 kv dims per core
NQT = 512                       # query tile (free dim per PSUM bank)
NKC = 128                       # key chunk (partition dim)

_cache = {}
LAST_RESULT = None              # BassKernelResults of the most recent run


def plan_mask(mask, T):
    """Classify (key-chunk i, q-tile j) tiles of exp(mask).T.

    Returns (plans, emt_tiles): plans[j] = list of (i, kind, emt_idx) where
    kind 0 = no mask needed (exp(mask)==1 on tile), kind 1 = multiply by
    emt_tiles[emt_idx].  All-zero tiles are skipped entirely (they contribute
    nothing to P@V nor to the softmax denominator).
    """
    m = np.asarray(mask, dtype=np.float32).reshape(T, T)
    with np.errstate(under="ignore"):
        em = np.exp(m).T.astype(np.float32)   # em[k, q] = exp(mask[q, k])
    nj, nk = T // NQT, T // NKC
    plans, emt_tiles = [], []
    for j in range(nj):
        pj = []
        for i in range(nk):
            t = em[i * NKC:(i + 1) * NKC, j * NQT:(j + 1) * NQT]
            if not t.any():
                continue
            if (t == 1.0).all():
                pj.append((i, 0, -1))
            else:
                pj.append((i, 1, len(emt_tiles)))
                emt_tiles.append(t.astype(BF16))
        plans.append(pj)
    return plans, emt_tiles


def build_graph(T, plans, n_emt):
    """Build the SPMD Bacc graph (same on all 8 cores; shards arrive as data)."""
    import concourse.bass as bass  # noqa: F401
    import concourse.mybir as mybir
    import concourse.tile as tile
    from concourse import bacc, masks

    f32, bf16 = mybir.dt.float32, mybir.dt.bfloat16
    AF, ALU = mybir.ActivationFunctionType, mybir.AluOpType

    nj = T // NQT          # q tiles
    nhc = HID // 128       # contraction chunks over hidden dim (16)
    noc = (HQ * D) // 128  # contraction chunks over gathered head dim (16)
    nem = max(n_emt, 1)
    assert nj == 4, "schedule below is specialized for T=2048"

    nc = bacc.Bacc("TRN2", target_bir_lowering=False, debug=False,
                   num_devices=NCORES)

    # host-prepped layouts: every tensor reads contiguous >=4KB per partition
    xt = nc.dram_tensor("xt", [nj, 128, nhc, NQT], bf16,
                        kind="ExternalInput").ap()
    wqt = nc.dram_tensor("wqt", [128, nhc, QDIM], bf16,
                         kind="ExternalInput").ap()
    wkt = nc.dram_tensor("wkt", [128, nhc, KVDIM], bf16,
                         kind="ExternalInput").ap()
    wvt = nc.dram_tensor("wvt", [128, nhc, KVDIM], bf16,
                         kind="ExternalInput").ap()
    qb = nc.dram_tensor("qb", [128, 4], f32, kind="ExternalInput").ap()
    kb = nc.dram_tensor("kb", [128, 1], f32, kind="ExternalInput").ap()
    vb = nc.dram_tensor("vb", [1, KVDIM], bf16, kind="ExternalInput").ap()
    cosq = nc.dram_tensor("cosq", [128, T], f32, kind="ExternalInput").ap()
    ssin = nc.dram_tensor("ssin", [128, T], f32, kind="ExternalInput").ap()
    emt = nc.dram_tensor("emt", [128, nem, NQT], bf16,
                         kind="ExternalInput").ap()
    owt = nc.dram_tensor("owt", [128, noc, QDIM], bf16,
                         kind="ExternalInput").ap()
    out = nc.dram_tensor("out", [T, QDIM], bf16, kind="ExternalOutput").ap()

    rg = [[0, 1, 2, 3], [4, 5, 6, 7]]

    with tile.TileContext(nc) as tc:
        with tc.tile_pool(name="dramp", bufs=1, space="DRAM") as dramp:
            ag_in = [[dramp.tile([128, NQT], bf16, name=f"agin{j}_{m}")
                      for m in range(4)] for j in range(nj)]
            ag_out = [[dramp.tile([NGROUPS * 128, NQT], bf16,
                                  name=f"agout{j}_{m}")
                       for m in range(4)] for j in range(nj)]
            dum_in = dramp.tile([1, 128], bf16, name="dum_in")
            dum_out = dramp.tile([4, 128], bf16, name="dum_out")

        with tc.tile_pool(name="persist", bufs=1) as pp:
            # Q^T per head-pair chunk: [128 (2 heads x 64), T]
            qt = [pp.tile([128, T], bf16, name=f"qt{m}") for m in range(4)]
            # K^T duplicated per kv head: [128 = kv dup'd twice, T]
            ktd = [pp.tile([128, T], bf16, name=f"ktd{k}") for k in range(2)]
            # V per key chunk: [128 keys, 130] (V0|one|V1|one)
            vsb = [pp.tile([128, 130], bf16, name=f"v{i}")
                   for i in range(T // NKC)]
            ow_sb = pp.tile([128, noc, QDIM], bf16, name="ow_sb")
            wqq = [pp.tile([128, 4, QDIM], bf16, name=f"wqq{r}")
                   for r in range(4)]
            wk_sb = pp.tile([128, nhc, KVDIM], bf16, name="wk_sb")
            wv_sb = pp.tile([128, nhc, KVDIM], bf16, name="wv_sb")
            cos_sb = pp.tile([128, T], f32, name="cos_sb")
            ssin_sb = pp.tile([128, T], f32, name="ssin_sb")
            emt_sb = pp.tile([128, nem, NQT], bf16, name="emt_sb")
            ident = pp.tile([128, 128], bf16, name="ident")
            vb_sb = pp.tile([1, KVDIM], bf16, name="vb_sb")
            ones_row = pp.tile([1, NQT], bf16, name="ones_row")
            qb_sb = pp.tile([128, 4], f32, name="qb_sb")
            kb_sb = pp.tile([128, 1], f32, name="kb_sb")

            nc.sync.dma_start(out=qb_sb[:], in_=qb)
            nc.sync.dma_start(out=kb_sb[:], in_=kb)
            nc.sync.dma_start(out=vb_sb[:], in_=vb)
            nc.vector.memset(ones_row[:], 1.0)
            # touch Exp once so the ACT table load (~2.7us) happens during
            # the startup DMA wait, not before the first real softmax
            warm = pp.tile([1, 2], bf16, name="warm")
            nc.scalar.activation(warm[:], ones_row[0:1, 0:2], AF.Exp,
                                 scale=0.125)
            # dummy gather: absorbs the one-time collective barrier/warm-up
            # (~12us trigger delay + ~2.5x duration) before the first real one
            nc.sync.dma_start(out=dum_in[:], in_=ones_row[0:1, 0:128])
            nc.gpsimd.collective_compute(
                "AllGather", ALU.bypass, replica_groups=rg,
                ins=[dum_in.opt()], outs=[dum_out.opt()])
            masks.make_identity(nc, ident[:])

            with tc.tile_pool(name="projx", bufs=2) as px, \
                 tc.tile_pool(name="projtmp", bufs=2) as ptmp, \
                 tc.tile_pool(name="pmain", bufs=2, space="PSUM") as pmain, \
                 tc.tile_pool(name="psS", bufs=2, space="PSUM") as psS, \
                 tc.tile_pool(name="psO", bufs=1, space="PSUM") as psO, \
                 tc.tile_pool(name="ptp", bufs=3) as ptp, \
                 tc.tile_pool(name="evp", bufs=2) as evp, \
                 tc.tile_pool(name="agp", bufs=8) as agp, \
                 tc.tile_pool(name="outp", bufs=2) as outp:

                x_tiles = {}

                def load_x_quarter(tb, qr):
                    xq = px.tile([128, 4, NQT], bf16, name="x_sb", tag="x_sb",
                                 bufs=8)
                    nc.sync.dma_start(out=xq[:],
                                      in_=xt[tb, :, qr * 4:(qr + 1) * 4, :])
                    x_tiles.setdefault(tb, []).append(xq)

                # startup: cos/sin FIRST — they gate the RoPE evict chain,
                # which gates the PSUM-pool rotation of the whole projection
                # phase.  Then x(0) quarters interleaved with wq quarters so
                # the first Q matmuls start after ~3MB instead of ~7MB.
                nc.sync.dma_start(out=cos_sb[:], in_=cosq)
                nc.sync.dma_start(out=ssin_sb[:], in_=ssin)
                for qr in range(4):
                    load_x_quarter(0, qr)
                    nc.sync.dma_start(out=wqq[qr][:],
                                      in_=wqt[:, qr * 4:(qr + 1) * 4, :])
                nc.sync.dma_start(out=wk_sb[:], in_=wkt)
                nc.sync.dma_start(out=wv_sb[:], in_=wvt)
                nc.sync.dma_start(out=emt_sb[:], in_=emt)
                nc.sync.dma_start(out=ow_sb[:], in_=owt)

                def rope_evict(ps, bias_col, dst, ts):
                    """dst = RoPE(ps + bias) cast to bf16."""
                    t2 = ptmp.tile([128, NQT], f32, name="t2", tag="t2")
                    nc.vector.scalar_tensor_tensor(
                        t2[:], ps[:], bias_col, ssin_sb[:, ts],
                        op0=ALU.add, op1=ALU.mult)
                    t2s = ptmp.tile([128, NQT], f32, name="t2s", tag="t2s")
                    for blk in range(4):
                        sb = blk ^ 1
                        # off the sync queue: keeps it free for attention
                        nc.gpsimd.dma_start(
                            out=t2s[blk * 32:(blk + 1) * 32, :],
                            in_=t2[sb * 32:(sb + 1) * 32, :])
                    t1 = ptmp.tile([128, NQT], f32, name="t1", tag="t1")
                    nc.vector.scalar_tensor_tensor(
                        t1[:], ps[:], bias_col, cos_sb[:, ts],
                        op0=ALU.add, op1=ALU.mult)
                    nc.vector.tensor_add(dst, t1[:], t2s[:])

                def proj_steps(tb):
                    """Projection of T-block tb as ~1us micro-steps."""
                    ts = slice(tb * NQT, (tb + 1) * NQT)
                    state = {}

                    def qs(m, qr):
                        def f():
                            xq = x_tiles[tb]
                            if qr == 0:
                                state[m] = pmain.tile([128, NQT], f32,
                                                      name="ps", tag="ps")
                            ps = state[m]
                            for c in range(qr * 4, qr * 4 + 4):
                                nc.tensor.matmul(
                                    ps[:],
                                    wqq[c // 4][:, c % 4,
                                                m * 128:(m + 1) * 128],
                                    xq[c // 4][:, c % 4, :],
                                    start=(c == 0), stop=(c == nhc - 1))
                            if qr == 3:
                                rope_evict(ps, qb_sb[:, m:m + 1],
                                           qt[m][:, ts], ts)
                        return f

                    def ks(qr):
                        def f():
                            xq = x_tiles[tb]
                            if qr == 0:
                                state['k'] = pmain.tile([128, NQT], f32,
                                                        name="psk", tag="ps")
                            psk = state['k']
                            for c in range(qr * 4, qr * 4 + 4):
                                nc.tensor.matmul(psk[:], wk_sb[:, c, :],
                                                 xq[c // 4][:, c % 4, :],
                                                 start=(c == 0),
                                                 stop=(c == nhc - 1))
                            if qr == 3:
                                kf = ptmp.tile([128, NQT], bf16, name="kf",
                                               tag="kf")
                                rope_evict(psk, kb_sb[:, 0:1], kf[:], ts)
                                for half in (0, 1):
                                    for dsth in (0, 1):
                                        nc.gpsimd.dma_start(
                                            out=ktd[half][dsth * 64:
                                                          (dsth + 1) * 64, ts],
                                            in_=kf[half * 64:(half + 1) * 64,
                                                   :])
                        return f

                    def vs(qr):
                        # V^T: kv dims on partitions, T on free — weights
                        # stationary, N=512 streams.
                        def f():
                            xq = x_tiles[tb]
                            if qr == 0:
                                state['v'] = pmain.tile([128, NQT], f32,
                                                        name="psv", tag="ps")
                            psv = state['v']
                            for c in range(qr * 4, qr * 4 + 4):
                                nc.tensor.matmul(psv[:], wv_sb[:, c, :],
                                                 xq[c // 4][:, c % 4, :],
                                                 start=(c == 0), stop=False)
                            if qr == 3:
                                nc.tensor.matmul(psv[:], vb_sb[:],
                                                 ones_row[:],
                                                 start=False, stop=True)
                                vt = ptmp.tile([128, NQT], bf16, name="vt",
                                               tag="vt")
                                nc.vector.tensor_copy(vt[:], psv[:])
                                state['vt'] = vt
                        return f

                    def vtr():
                        # PE-transpose V^T back to [keys, dims] for P@V.
                        vt = state['vt']
                        pstr = pmain.tile([128, NQT], bf16, name="pstr",
                                          tag="ps")
                        for ti in range(4):
                            nc.tensor.transpose(
                                pstr[:, ti * 128:(ti + 1) * 128],
                                vt[:, ti * 128:(ti + 1) * 128], ident[:])
                        for ti in range(4):
                            vi = tb * 4 + ti
                            nc.vector.memset(vsb[vi][:, 64:65], 1.0)
                            nc.vector.memset(vsb[vi][:, 129:130], 1.0)
                            nc.vector.tensor_copy(
                                vsb[vi][:, 0:64],
                                pstr[:, ti * 128:ti * 128 + 64])
                            nc.vector.tensor_copy(
                                vsb[vi][:, 65:129],
                                pstr[:, ti * 128 + 64:(ti + 1) * 128])

                    # q0/K/V first: attention(tb) head-pair 0 needs exactly
                    # these, so its score->exp stream starts ~20us earlier
                    # than with the m-ordered emission.
                    steps = [lambda qr=qr: load_x_quarter(tb, qr)
                             for qr in range(4)] if tb > 0 else []
                    steps += [qs(0, qr) for qr in range(4)]
                    steps += [ks(qr) for qr in range(4)]
                    steps += [vs(qr) for qr in range(4)]
                    steps.append(vtr)
                    for m in range(1, 4):
                        steps += [qs(m, qr) for qr in range(4)]
                    return steps

                ag_tiles = {}

                def load_ag(j, m):
                    def f():
                        ag_sbm = agp.tile([128, NGROUPS, NQT], bf16,
                                          name="ag_sb")
                        nc.sync.dma_start(
                            out=ag_sbm[:],
                            in_=ag_out[j][m].rearrange("(g p) t -> p g t",
                                                       p=128))
                        ag_tiles[(j, m)] = ag_sbm
                    return f

                def oproj_evict(j, tt, pf):
                    ot = outp.tile([128, QDIM], bf16, name="ot", tag="ot")
                    nc.vector.tensor_copy(ot[:], pf[:])
                    nc.sync.dma_start(
                        out=out[j * NQT + tt * 128:
                                j * NQT + (tt + 1) * 128, :],
                        in_=ot[:])

                def oproj_steps(j):
                    state = {}

                    def os(tt, m):
                        def f():
                            if m == 0:
                                state[tt] = pmain.tile([128, QDIM], f32,
                                                       name="pf", tag="ps")
                            pf = state[tt]
                            agm = ag_tiles[(j, m)]
                            for g in range(NGROUPS):
                                nc.tensor.matmul(
                                    pf[:],
                                    agm[:, g, tt * 128:(tt + 1) * 128],
                                    ow_sb[:, g * 4 + m, :],
                                    start=(m == 0 and g == 0),
                                    stop=(m == 3 and g == NGROUPS - 1))
                            if m == 3:
                                oproj_evict(j, tt, pf)
                        return f

                    return [os(tt, m) for tt in range(4) for m in range(4)]

                def attn_hp(j, hp, filler):
                    qs_ = slice(j * NQT, (j + 1) * NQT)
                    kv = hp // 2
                    po0 = psO.tile([65, NQT], f32, name="po0", tag="po0")
                    po1 = psO.tile([65, NQT], f32, name="po1", tag="po1")
                    ch = plans[j]
                    n_ch = len(ch)
                    pss_t = {}

                    def scores(ci):
                        i = ch[ci][0]
                        pss = psS.tile([128, 1024], f32, name="pss",
                                       tag="pss")
                        # head-lo on array rows 0:64, head-hi on 64:128 —
                        # concurrent row-groups, separate PSUM banks
                        nc.tensor.matmul(
                            pss[:, 0:512],
                            ktd[kv][0:64, i * NKC:(i + 1) * NKC],
                            qt[hp][0:64, qs_], start=True, stop=True)
                        nc.tensor.matmul(
                            pss[:, 512:1024],
                            ktd[kv][64:128, i * NKC:(i + 1) * NKC],
                            qt[hp][64:128, qs_], start=True, stop=True)
                        pss_t[ci] = pss

                    scores(0)
                    for ci in range(n_ch):
                        if ci + 1 < n_ch:
                            scores(ci + 1)
                        i, kind, gi = ch[ci]
                        pss = pss_t.pop(ci)
                        pt = ptp.tile([128, 1024], bf16, name="pt", tag="pt")
                        nc.scalar.activation(pt[:], pss[:], AF.Exp,
                                             scale=0.125)
                        if kind == 1:
                            nc.vector.tensor_mul(pt[:, 0:512], pt[:, 0:512],
                                                 emt_sb[:, gi, :])
                            nc.vector.tensor_mul(pt[:, 512:1024],
                                                 pt[:, 512:1024],
                                                 emt_sb[:, gi, :])
                        # ~1us micro-steps of other PE work, emitted BETWEEN
                        # this chunk's scores and its P@V: the PE queue never
                        # head-of-line-stalls on the exp wait, which both
                        # fills the gap and keeps the HAM activity monitor
                        # seeing a busy PE (K=8/8, full clock).  Double pull
                        # on the first chunk: covers the previous head-pair's
                        # softmax-evict chain (the new po accumulators wait
                        # on its PSUM copies).
                        filler.tick()
                        if ci == 0:
                            filler.tick()
                        vsl = (vsb[i][:, 0:65] if kv == 0
                               else vsb[i][:, 65:130])
                        nc.tensor.matmul(po0[:], vsl, pt[:, 0:512],
                                         start=(ci == 0),
                                         stop=(ci == n_ch - 1))
                        nc.tensor.matmul(po1[:], vsl, pt[:, 512:1024],
                                         start=(ci == 0),
                                         stop=(ci == n_ch - 1))
                    at = evp.tile([128, NQT], bf16, name="at", tag="at",
                                  bufs=3)
                    for s, po in enumerate((po0, po1)):
                        # One fast copy frees the PSUM bank; the divide chain
                        # then runs off the PE critical path from SBUF.
                        pocp = evp.tile([65, NQT], f32, name="pocp",
                                        tag="pocp")
                        nc.vector.tensor_copy(pocp[:], po[:])
                        # reciprocal is ~6 cycles/elem serial per partition:
                        # spread the 512 sums over 128 lanes via two small
                        # DMAs so it costs ~0.2us instead of 1.7us
                        rs = evp.tile([128, 4], f32, name="rs", tag="rs")
                        nc.gpsimd.dma_start(out=rs[:], in_=pocp[64:65, :])
                        rr = evp.tile([128, 4], f32, name="rr", tag="rr")
                        nc.vector.reciprocal(rr[:], rs[:])
                        rc = evp.tile([1, NQT], f32, name="rc", tag="rc")
                        nc.gpsimd.dma_start(out=rc[:], in_=rr[:])
                        rb = evp.tile([64, NQT], f32, name="rb", tag="rb")
                        nc.gpsimd.partition_broadcast(rb[:], rc[:])
                        nc.vector.tensor_mul(at[s * 64:(s + 1) * 64, :],
                                             pocp[0:64, :], rb[:])
                    nc.sync.dma_start(out=ag_in[j][hp][:], in_=at[:])
                    nc.gpsimd.collective_compute(
                        "AllGather", ALU.bypass, replica_groups=rg,
                        ins=[ag_in[j][hp].opt()], outs=[ag_out[j][hp].opt()])

                def interleave(a, b, period=2):
                    """merge list b into list a, one b-item every `period`."""
                    out_, bi = [], 0
                    for k, u in enumerate(a):
                        out_.append(u)
                        if k % period == period - 1 and bi < len(b):
                            out_.append(b[bi])
                            bi += 1
                    out_ += b[bi:]
                    return out_

                class Pacer:
                    """Spread filler steps evenly over a window's chunk
                    slots so no stretch of attention runs bare (exp-bound
                    with an idling, HAM-cooling PE)."""

                    def __init__(self, steps, slots):
                        self.steps, self.slots = steps, max(slots, 1)
                        self.i, self.seen = 0, 0

                    def tick(self):
                        self.seen += 1
                        want = (len(self.steps) * self.seen) // self.slots
                        while self.i < min(want, len(self.steps)):
                            self.steps[self.i]()
                            self.i += 1

                    def drain(self):
                        while self.i < len(self.steps):
                            self.steps[self.i]()
                            self.i += 1

                # ---------------- the schedule ----------------
                for u in proj_steps(0):
                    u()
                for j in range(nj):
                    steps = []
                    if j == 0:
                        steps = proj_steps(1)
                    elif j == 1:
                        steps = proj_steps(2)
                    elif j == 2:
                        steps = interleave(proj_steps(3),
                                           [load_ag(0, m) for m in range(4)] +
                                           [load_ag(1, m) for m in range(4)])
                    else:
                        # window 3 is exp-heavy (~85us) with 64 chunk slots:
                        # all three earlier o_proj tiles fill it.  (2,m) ag
                        # buffers only free up as o_proj(0) retires (0,m).
                        steps = oproj_steps(0)
                        steps += [load_ag(2, m) for m in range(4)]
                        steps += oproj_steps(1)
                        steps += oproj_steps(2)
                    pacer = Pacer(steps, 4 * len(plans[j]))
                    for hp in range(4):
                        attn_hp(j, hp, pacer)
                    pacer.drain()
                # tail: last q-tile's o_proj accumulates m-major so only the
                # final quarter waits on the last (small) AllGather.  ALL of
                # the last tile's gather loads go here — a load emitted
                # mid-window waits on its gather ON THE SYNC QUEUE and
                # head-of-line-blocks the at->gather-input DMAs behind it,
                # cascading the remaining gathers ~25us late.
                jl = nj - 1
                for m in range(4):
                    load_ag(jl, m)()
                pf = [pmain.tile([128, QDIM], f32, name="pf", tag="ps")
                      for _ in range(2)] + \
                     [psS.tile([128, QDIM], f32, name="pf2", tag="pss")
                      for _ in range(2)]
                for m in range(4):
                    agm = ag_tiles[(jl, m)]
                    for tt in range(4):
                        for g in range(NGROUPS):
                            nc.tensor.matmul(
                                pf[tt][:],
                                agm[:, g, tt * 128:(tt + 1) * 128],
                                ow_sb[:, g * 4 + m, :],
                                start=(m == 0 and g == 0),
                                stop=(m == 3 and g == NGROUPS - 1))
                for tt in range(4):
                    oproj_evict(jl, tt, pf[tt])

    nc.compile()
    return nc


def prep_inputs(hidden, positions, mask, q_w, q_b, k_w, k_b, v_w, v_b, o_w,
                emt_tiles):
    """Host-side shard + transform -> in_maps for the 8 cores."""
    B, T, _ = hidden.shape
    nhc = HID // 128
    pos = np.asarray(positions)[0].astype(np.float32)
    inv_freq = (1.0 / (THETA ** (np.arange(0, D, 2, dtype=np.float32) / D)))
    freqs = pos[:, None] * inv_freq[None, :]          # (T, 32)
    cos_t, sin_t = np.cos(freqs).T, np.sin(freqs).T   # (32, T)
    cos_tab = np.ascontiguousarray(np.tile(cos_t, (4, 1)), dtype=np.float32)
    ssin_tab = np.ascontiguousarray(
        np.concatenate([sin_t, -sin_t, sin_t, -sin_t], axis=0),
        dtype=np.float32)

    if emt_tiles:
        emt_arr = np.ascontiguousarray(
            np.stack(emt_tiles, axis=1)).astype(BF16)  # [128, n_emt, 512]
    else:
        emt_arr = np.zeros((128, 1, NQT), BF16)

    def chunked(w):   # [HID, O] -> [128, nhc, O]
        return np.ascontiguousarray(
            w.reshape(nhc, 128, w.shape[1]).transpose(1, 0, 2)).astype(BF16)

    nj = T // NQT
    xts = []
    for b in range(B):
        a = np.asarray(hidden[b], np.float32)          # [T, HID]
        a = a.reshape(nj, NQT, nhc, 128)               # [tb, u, c, p]
        xts.append(np.ascontiguousarray(
            a.transpose(0, 3, 2, 1)).astype(BF16))     # [tb, p, c, u]

    in_maps = []
    for c in range(NCORES):
        b, g = c // NGROUPS, c % NGROUPS
        qsl = slice(QDIM * g, QDIM * (g + 1))
        ksl = slice(KVDIM * g, KVDIM * (g + 1))
        in_maps.append({
            "xt": xts[b],
            "wqt": chunked(np.asarray(q_w[qsl, :]).T),
            "wkt": chunked(np.asarray(k_w[ksl, :]).T),
            "wvt": chunked(np.asarray(v_w[ksl, :]).T),
            "qb": np.ascontiguousarray(
                np.asarray(q_b[qsl], np.float32).reshape(4, 128).T),
            "kb": np.ascontiguousarray(
                np.asarray(k_b[ksl], np.float32).reshape(1, 128).T),
            "vb": np.asarray(v_b[ksl]).astype(BF16).reshape(1, KVDIM),
            "cosq": cos_tab,
            "ssin": ssin_tab,
            "emt": emt_arr,
            "owt": chunked(np.asarray(o_w).T[:, qsl]),
        })
    return in_maps


def _ensure_ntff_hook():
    """Provide antenv.axon_hooks in containers whose antenv stub lacks it,
    wiring the ctypes NTFF profiler from the injected axon boot package."""
    import sys
    import types
    try:
        from antenv.axon_hooks import get_axon_ntff_profile_hook  # noqa: F401
        return True
    except ImportError:
        pass
    try:
        import antenv
        from trn_agent_boot.trn_boot import _ntff_profile_via_ctypes
        hook = _ntff_profile_via_ctypes("/opt/axon/libaxon_pjrt.so")
        if hook is None:
            return False
        mod = types.ModuleType("antenv.axon_hooks")
        state = {"h": hook}
        mod.get_axon_ntff_profile_hook = lambda: state["h"]
        mod.set_axon_ntff_profile_hook = lambda h: state.__setitem__("h", h)
        sys.modules["antenv.axon_hooks"] = mod
        antenv.axon_hooks = mod
        return True
    except Exception:
        return False


def kernel(hidden, positions, mask, q_w, q_b, k_w, k_b, v_w, v_b, o_w):
    global LAST_RESULT
    from concourse import bass_utils

    hidden = np.asarray(hidden)
    B, T, _ = hidden.shape
    mask_key = (T, hash(np.asarray(mask).tobytes()))
    if mask_key not in _cache:
        plans, emt_tiles = plan_mask(mask, T)
        nc = build_graph(T, plans, len(emt_tiles))
        _cache[mask_key] = (nc, emt_tiles)
    nc, emt_tiles = _cache[mask_key]

    in_maps = prep_inputs(hidden, positions, mask, q_w, q_b, k_w, k_b,
                          v_w, v_b, o_w, emt_tiles)
    trace = os.environ.get("BASS_KERNEL_TRACE", "0") == "1"
    if trace:
        trace = _ensure_ntff_hook()
    res = bass_utils.run_bass_kernel_spmd(nc, in_maps,
                                          core_ids=list(range(NCORES)),
                                          trace=trace)
    LAST_RESULT = res
    out = np.zeros((B, T, HID), np.float32)
    for c in range(NCORES):
        b, g = c // NGROUPS, c % NGROUPS
        out[b, :, QDIM * g:QDIM * (g + 1)] = \
            res.results[c]["out"].astype(np.float32)
    return out


# revision 21
# speedup vs baseline: 1.0151x; 1.0090x over previous
"""GQA attention (32 q heads / 8 kv heads, D=64, HID=2048, B=2, T=2048)
distributed over 8 TRN2 NeuronCores.

Sharding: 2-way data parallel (batch) x 4-way tensor parallel (head groups).
Core c handles batch c//4 and head group g=c%4 (q heads [8g,8g+8), kv heads
[2g,2g+2)).  Each core projects Q^T/K^T (transposed layout: head-dims on
partitions, T on free axis), computes V^T the same cheap way (weights
stationary, N=512 streams) and PE-transposes it into the [keys, dims] layout
that P@V needs.  Scores^T = K @ Q^T per head with keys on partitions, exp via
ScalarE (no max-subtraction needed at these magnitudes; masked entries
multiply to exactly 0 by a host-precomputed exp(mask) factor), then
out^T = Vext^T @ P^T where Vext carries a ones column producing the softmax
denominators for free.

The attention phase is ScalarE(exp)-bound (~1.33us per key-chunk vs ~0.78us
of matmul), so all other PE work — next block's projections, o_proj of
previous q-tiles, gather loads — is chopped into ~1us micro-steps and
interleaved ONE PER KEY-CHUNK into the attention emission, with the score
matmuls software-pipelined one chunk ahead so the exp stream never waits.
Attention outputs are AllGathered per head-pair (16 gathers of 128KB); a
dummy gather at t=0 absorbs the collective warm-up, and the last q-tile's
o_proj accumulates m-major so only its final quarter waits on the last
gather.  All host-side layouts give every DMA >=4KB contiguous per partition
(hardware DGE fast path).
"""

import os
import numpy as np
import ml_dtypes

BF16 = ml_dtypes.bfloat16

HQ, HKV, D, HID, THETA = 32, 8, 64, 2048, 10000.0
NCORES, NGROUPS = 8, 4
QDIM = HQ * D // NGROUPS        # 512 q dims per core
KVDIM = HKV * D // NGROUPS      # 128 kv dims per core
NQT = 512                       # query tile (free dim per PSUM bank)
NKC = 128                       # key chunk (partition dim)

_cache = {}
LAST_RESULT = None              # BassKernelResults of the most recent run


def plan_mask(mask, T):
    """Classify (key-chunk i, q-tile j) tiles of exp(mask).T.

    Returns (plans, emt_tiles): plans[j] = list of (i, kind, emt_idx) where
    kind 0 = no mask needed (exp(mask)==1 on tile), kind 1 = multiply by
    emt_tiles[emt_idx].  All-zero tiles are skipped entirely (they contribute
    nothing to P@V nor to the softmax denominator).
    """
    m = np.asarray(mask, dtype=np.float32).reshape(T, T)
    with np.errstate(under="ignore"):
        em = np.exp(m).T.astype(np.float32)   # em[k, q] = exp(mask[q, k])
    nj, nk = T // NQT, T // NKC
    plans, emt_tiles = [], []
    for j in range(nj):
        pj = []
        for i in range(nk):
            t = em[i * NKC:(i + 1) * NKC, j * NQT:(j + 1) * NQT]
            if not t.any():
                continue
            if (t == 1.0).all():
                pj.append((i, 0, -1))
            else:
                pj.append((i, 1, len(emt_tiles)))
                emt_tiles.append(t.astype(BF16))
        plans.append(pj)
    return plans, emt_tiles


def build_graph(T, plans, n_emt):
    """Build the SPMD Bacc graph (same on all 8 cores; shards arrive as data)."""
    import concourse.bass as bass  # noqa: F401
    import concourse.mybir as mybir
    import concourse.tile as tile
    from concourse import bacc, masks

    f32, bf16 = mybir.dt.float32, mybir.dt.bfloat16
    AF, ALU = mybir.ActivationFunctionType, mybir.AluOpType

    nj = T // NQT          # q tiles
    nhc = HID // 128       # contraction chunks over hidden dim (16)
    noc = (HQ * D) // 128  # contraction chunks over gathered head dim (16)
    nem = max(n_emt, 1)
    assert nj == 4, "schedule below is specialized for T=2048"

    nc = bacc.Bacc("TRN2", target_bir_lowering=False, debug=False,
                   num_devices=NCORES)

    # host-prepped layouts: every tensor reads contiguous >=4KB per partition
    xt = nc.dram_tensor("xt", [nj, 128, nhc, NQT], bf16,
                        kind="ExternalInput").ap()
    wqt = nc.dram_tensor("wqt", [128, nhc, QDIM], bf16,
                         kind="ExternalInput").ap()
    wkt = nc.dram_tensor("wkt", [128, nhc, KVDIM], bf16,
                         kind="ExternalInput").ap()
    wvt = nc.dram_tensor("wvt", [128, nhc, KVDIM], bf16,
                         kind="ExternalInput").ap()
    qb = nc.dram_tensor("qb", [128, 4], f32, kind="ExternalInput").ap()
    kb = nc.dram_tensor("kb", [128, 1], f32, kind="ExternalInput").ap()
    vb = nc.dram_tensor("vb", [1, KVDIM], bf16, kind="ExternalInput").ap()
    cosq = nc.dram_tensor("cosq", [128, T], f32, kind="ExternalInput").ap()
    ssin = nc.dram_tensor("ssin", [128, T], f32, kind="ExternalInput").ap()
    emt = nc.dram_tensor("emt", [128, nem, NQT], bf16,
                         kind="ExternalInput").ap()
    owt = nc.dram_tensor("owt", [128, noc, QDIM], bf16,
                         kind="ExternalInput").ap()
    out = nc.dram_tensor("out", [T, QDIM], bf16, kind="ExternalOutput").ap()

    rg = [[0, 1, 2, 3], [4, 5, 6, 7]]

    with tile.TileContext(nc) as tc:
        with tc.tile_pool(name="dramp", bufs=1, space="DRAM") as dramp:
            ag_in = [[dramp.tile([128, NQT], bf16, name=f"agin{j}_{m}")
                      for m in range(4)] for j in range(nj)]
            ag_out = [[dramp.tile([NGROUPS * 128, NQT], bf16,
                                  name=f"agout{j}_{m}")
                       for m in range(4)] for j in range(nj)]
            dum_in = dramp.tile([1, 128], bf16, name="dum_in")
            dum_out = dramp.tile([4, 128], bf16, name="dum_out")

        with tc.tile_pool(name="persist", bufs=1) as pp:
            # Q^T per head-pair chunk: [128 (2 heads x 64), T]
            qt = [pp.tile([128, T], bf16, name=f"qt{m}") for m in range(4)]
            # K^T duplicated per kv head: [128 = kv dup'd twice, T]
            ktd = [pp.tile([128, T], bf16, name=f"ktd{k}") for k in range(2)]
            # V per key chunk: [128 keys, 130] (V0|one|V1|one)
            vsb = [pp.tile([128, 130], bf16, name=f"v{i}")
                   for i in range(T // NKC)]
            ow_sb = pp.tile([128, noc, QDIM], bf16, name="ow_sb")
            wqq = [pp.tile([128, 4, QDIM], bf16, name=f"wqq{r}")
                   for r in range(4)]
            wk_sb = pp.tile([128, nhc, KVDIM], bf16, name="wk_sb")
            wv_sb = pp.tile([128, nhc, KVDIM], bf16, name="wv_sb")
            cos_sb = pp.tile([128, T], f32, name="cos_sb")
            ssin_sb = pp.tile([128, T], f32, name="ssin_sb")
            emt_sb = pp.tile([128, nem, NQT], bf16, name="emt_sb")
            ident = pp.tile([128, 128], bf16, name="ident")
            vb_sb = pp.tile([1, KVDIM], bf16, name="vb_sb")
            ones_row = pp.tile([1, NQT], bf16, name="ones_row")
            qb_sb = pp.tile([128, 4], f32, name="qb_sb")
            kb_sb = pp.tile([128, 1], f32, name="kb_sb")

            nc.sync.dma_start(out=qb_sb[:], in_=qb)
            nc.sync.dma_start(out=kb_sb[:], in_=kb)
            nc.sync.dma_start(out=vb_sb[:], in_=vb)
            nc.vector.memset(ones_row[:], 1.0)
            # touch Exp once so the ACT table load (~2.7us) happens during
            # the startup DMA wait, not before the first real softmax
            warm = pp.tile([1, 2], bf16, name="warm")
            nc.scalar.activation(warm[:], ones_row[0:1, 0:2], AF.Exp,
                                 scale=0.125)
            # dummy gather: absorbs the one-time collective barrier/warm-up
            # (~12us trigger delay + ~2.5x duration) before the first real one
            nc.sync.dma_start(out=dum_in[:], in_=ones_row[0:1, 0:128])
            nc.gpsimd.collective_compute(
                "AllGather", ALU.bypass, replica_groups=rg,
                ins=[dum_in.opt()], outs=[dum_out.opt()])
            masks.make_identity(nc, ident[:])

            with tc.tile_pool(name="projx", bufs=2) as px, \
                 tc.tile_pool(name="projtmp", bufs=2) as ptmp, \
                 tc.tile_pool(name="pmain", bufs=2, space="PSUM") as pmain, \
                 tc.tile_pool(name="psS", bufs=2, space="PSUM") as psS, \
                 tc.tile_pool(name="psO", bufs=1, space="PSUM") as psO, \
                 tc.tile_pool(name="ptp", bufs=3) as ptp, \
                 tc.tile_pool(name="evp", bufs=2) as evp, \
                 tc.tile_pool(name="agp", bufs=8) as agp, \
                 tc.tile_pool(name="outp", bufs=2) as outp:

                x_tiles = {}

                def load_x_quarter(tb, qr):
                    xq = px.tile([128, 4, NQT], bf16, name="x_sb", tag="x_sb",
                                 bufs=8)
                    nc.sync.dma_start(out=xq[:],
                                      in_=xt[tb, :, qr * 4:(qr + 1) * 4, :])
                    x_tiles.setdefault(tb, []).append(xq)

                # startup: first x/wq quarter (first matmuls at ~1MB), then
                # cos/sin (they gate the RoPE evict chain and with it the
                # PSUM-pool rotation of the whole projection), then the rest.
                load_x_quarter(0, 0)
                nc.sync.dma_start(out=wqq[0][:], in_=wqt[:, 0:4, :])
                nc.sync.dma_start(out=cos_sb[:], in_=cosq)
                nc.sync.dma_start(out=ssin_sb[:], in_=ssin)
                for qr in range(1, 4):
                    load_x_quarter(0, qr)
                    nc.sync.dma_start(out=wqq[qr][:],
                                      in_=wqt[:, qr * 4:(qr + 1) * 4, :])
                nc.sync.dma_start(out=wk_sb[:], in_=wkt)
                nc.sync.dma_start(out=wv_sb[:], in_=wvt)
                nc.sync.dma_start(out=emt_sb[:], in_=emt)
                nc.sync.dma_start(out=ow_sb[:], in_=owt)

                def rope_evict(ps, bias_col, dst, ts):
                    """dst = RoPE(ps + bias) cast to bf16."""
                    t2 = ptmp.tile([128, NQT], f32, name="t2", tag="t2")
                    nc.vector.scalar_tensor_tensor(
                        t2[:], ps[:], bias_col, ssin_sb[:, ts],
                        op0=ALU.add, op1=ALU.mult)
                    t2s = ptmp.tile([128, NQT], f32, name="t2s", tag="t2s")
                    for blk in range(4):
                        sb = blk ^ 1
                        # off the sync queue: keeps it free for attention
                        nc.gpsimd.dma_start(
                            out=t2s[blk * 32:(blk + 1) * 32, :],
                            in_=t2[sb * 32:(sb + 1) * 32, :])
                    t1 = ptmp.tile([128, NQT], f32, name="t1", tag="t1")
                    nc.vector.scalar_tensor_tensor(
                        t1[:], ps[:], bias_col, cos_sb[:, ts],
                        op0=ALU.add, op1=ALU.mult)
                    nc.vector.tensor_add(dst, t1[:], t2s[:])

                def proj_steps(tb):
                    """Projection of T-block tb as ~1us micro-steps."""
                    ts = slice(tb * NQT, (tb + 1) * NQT)
                    state = {}

                    def qs(m, qr):
                        def f():
                            xq = x_tiles[tb]
                            if qr == 0:
                                state[m] = pmain.tile([128, NQT], f32,
                                                      name="ps", tag="ps")
                            ps = state[m]
                            for c in range(qr * 4, qr * 4 + 4):
                                nc.tensor.matmul(
                                    ps[:],
                                    wqq[c // 4][:, c % 4,
                                                m * 128:(m + 1) * 128],
                                    xq[c // 4][:, c % 4, :],
                                    start=(c == 0), stop=(c == nhc - 1))
                            if qr == 3:
                                rope_evict(ps, qb_sb[:, m:m + 1],
                                           qt[m][:, ts], ts)
                        return f

                    def ks(qr):
                        def f():
                            xq = x_tiles[tb]
                            if qr == 0:
                                state['k'] = pmain.tile([128, NQT], f32,
                                                        name="psk", tag="ps")
                            psk = state['k']
                            for c in range(qr * 4, qr * 4 + 4):
                                nc.tensor.matmul(psk[:], wk_sb[:, c, :],
                                                 xq[c // 4][:, c % 4, :],
                                                 start=(c == 0),
                                                 stop=(c == nhc - 1))
                            if qr == 3:
                                kf = ptmp.tile([128, NQT], bf16, name="kf",
                                               tag="kf")
                                rope_evict(psk, kb_sb[:, 0:1], kf[:], ts)
                                for half in (0, 1):
                                    for dsth in (0, 1):
                                        nc.gpsimd.dma_start(
                                            out=ktd[half][dsth * 64:
                                                          (dsth + 1) * 64, ts],
                                            in_=kf[half * 64:(half + 1) * 64,
                                                   :])
                        return f

                    def vs(qr):
                        # V^T: kv dims on partitions, T on free — weights
                        # stationary, N=512 streams.
                        def f():
                            xq = x_tiles[tb]
                            if qr == 0:
                                state['v'] = pmain.tile([128, NQT], f32,
                                                        name="psv", tag="ps")
                            psv = state['v']
                            for c in range(qr * 4, qr * 4 + 4):
                                nc.tensor.matmul(psv[:], wv_sb[:, c, :],
                                                 xq[c // 4][:, c % 4, :],
                                                 start=(c == 0), stop=False)
                            if qr == 3:
                                nc.tensor.matmul(psv[:], vb_sb[:],
                                                 ones_row[:],
                                                 start=False, stop=True)
                                vt = ptmp.tile([128, NQT], bf16, name="vt",
                                               tag="vt")
                                nc.vector.tensor_copy(vt[:], psv[:])
                                state['vt'] = vt
                        return f

                    def vtr():
                        # PE-transpose V^T back to [keys, dims] for P@V.
                        vt = state['vt']
                        pstr = pmain.tile([128, NQT], bf16, name="pstr",
                                          tag="ps")
                        for ti in range(4):
                            nc.tensor.transpose(
                                pstr[:, ti * 128:(ti + 1) * 128],
                                vt[:, ti * 128:(ti + 1) * 128], ident[:])
                        for ti in range(4):
                            vi = tb * 4 + ti
                            nc.vector.memset(vsb[vi][:, 64:65], 1.0)
                            nc.vector.memset(vsb[vi][:, 129:130], 1.0)
                            nc.vector.tensor_copy(
                                vsb[vi][:, 0:64],
                                pstr[:, ti * 128:ti * 128 + 64])
                            nc.vector.tensor_copy(
                                vsb[vi][:, 65:129],
                                pstr[:, ti * 128 + 64:(ti + 1) * 128])

                    # q0/K/V first: attention(tb) head-pair 0 needs exactly
                    # these, so its score->exp stream starts ~20us earlier
                    # than with the m-ordered emission.
                    steps = [lambda qr=qr: load_x_quarter(tb, qr)
                             for qr in range(4)] if tb > 0 else []
                    steps += [qs(0, qr) for qr in range(4)]
                    steps += [ks(qr) for qr in range(4)]
                    steps += [vs(qr) for qr in range(4)]
                    steps.append(vtr)
                    for m in range(1, 4):
                        steps += [qs(m, qr) for qr in range(4)]
                    return steps

                ag_tiles = {}

                def load_ag(j, m):
                    def f():
                        ag_sbm = agp.tile([128, NGROUPS, NQT], bf16,
                                          name="ag_sb")
                        nc.sync.dma_start(
                            out=ag_sbm[:],
                            in_=ag_out[j][m].rearrange("(g p) t -> p g t",
                                                       p=128))
                        ag_tiles[(j, m)] = ag_sbm
                    return f

                def oproj_evict(j, tt, pf):
                    ot = outp.tile([128, QDIM], bf16, name="ot", tag="ot")
                    nc.vector.tensor_copy(ot[:], pf[:])
                    nc.sync.dma_start(
                        out=out[j * NQT + tt * 128:
                                j * NQT + (tt + 1) * 128, :],
                        in_=ot[:])

                def oproj_steps(j):
                    state = {}

                    def os(tt, m):
                        def f():
                            if m == 0:
                                state[tt] = pmain.tile([128, QDIM], f32,
                                                       name="pf", tag="ps")
                            pf = state[tt]
                            agm = ag_tiles[(j, m)]
                            for g in range(NGROUPS):
                                nc.tensor.matmul(
                                    pf[:],
                                    agm[:, g, tt * 128:(tt + 1) * 128],
                                    ow_sb[:, g * 4 + m, :],
                                    start=(m == 0 and g == 0),
                                    stop=(m == 3 and g == NGROUPS - 1))
                            if m == 3:
                                oproj_evict(j, tt, pf)
                        return f

                    return [os(tt, m) for tt in range(4) for m in range(4)]

                def attn_hp(j, hp, filler):
                    qs_ = slice(j * NQT, (j + 1) * NQT)
                    kv = hp // 2
                    po0 = psO.tile([65, NQT], f32, name="po0", tag="po0")
                    po1 = psO.tile([65, NQT], f32, name="po1", tag="po1")
                    ch = plans[j]
                    n_ch = len(ch)
                    pss_t = {}

                    def scores(ci):
                        i = ch[ci][0]
                        pss = psS.tile([128, 1024], f32, name="pss",
                                       tag="pss")
                        # head-lo on array rows 0:64, head-hi on 64:128 —
                        # concurrent row-groups, separate PSUM banks
                        nc.tensor.matmul(
                            pss[:, 0:512],
                            ktd[kv][0:64, i * NKC:(i + 1) * NKC],
                            qt[hp][0:64, qs_], start=True, stop=True)
                        nc.tensor.matmul(
                            pss[:, 512:1024],
                            ktd[kv][64:128, i * NKC:(i + 1) * NKC],
                            qt[hp][64:128, qs_], start=True, stop=True)
                        pss_t[ci] = pss

                    scores(0)
                    for ci in range(n_ch):
                        if ci + 1 < n_ch:
                            scores(ci + 1)
                        i, kind, gi = ch[ci]
                        pss = pss_t.pop(ci)
                        pt = ptp.tile([128, 1024], bf16, name="pt", tag="pt")
                        nc.scalar.activation(pt[:], pss[:], AF.Exp,
                                             scale=0.125)
                        if kind == 1:
                            nc.vector.tensor_mul(pt[:, 0:512], pt[:, 0:512],
                                                 emt_sb[:, gi, :])
                            nc.vector.tensor_mul(pt[:, 512:1024],
                                                 pt[:, 512:1024],
                                                 emt_sb[:, gi, :])
                        # ~1us micro-steps of other PE work, emitted BETWEEN
                        # this chunk's scores and its P@V: the PE queue never
                        # head-of-line-stalls on the exp wait, which both
                        # fills the gap and keeps the HAM activity monitor
                        # seeing a busy PE (K=8/8, full clock).  Double pull
                        # on the first chunk: covers the previous head-pair's
                        # softmax-evict chain (the new po accumulators wait
                        # on its PSUM copies).
                        filler.tick()
                        if ci == 0:
                            filler.tick()
                            filler.tick()
                        vsl = (vsb[i][:, 0:65] if kv == 0
                               else vsb[i][:, 65:130])
                        nc.tensor.matmul(po0[:], vsl, pt[:, 0:512],
                                         start=(ci == 0),
                                         stop=(ci == n_ch - 1))
                        nc.tensor.matmul(po1[:], vsl, pt[:, 512:1024],
                                         start=(ci == 0),
                                         stop=(ci == n_ch - 1))
                    at = evp.tile([128, NQT], bf16, name="at", tag="at",
                                  bufs=3)
                    for s, po in enumerate((po0, po1)):
                        # One fast copy frees the PSUM bank; the divide chain
                        # then runs off the PE critical path from SBUF.
                        pocp = evp.tile([65, NQT], f32, name="pocp",
                                        tag="pocp")
                        nc.vector.tensor_copy(pocp[:], po[:])
                        # reciprocal is ~6 cycles/elem serial per partition:
                        # spread the 512 sums over 128 lanes via two small
                        # DMAs so it costs ~0.2us instead of 1.7us
                        rs = evp.tile([128, 4], f32, name="rs", tag="rs")
                        nc.gpsimd.dma_start(out=rs[:], in_=pocp[64:65, :])
                        rr = evp.tile([128, 4], f32, name="rr", tag="rr")
                        nc.vector.reciprocal(rr[:], rs[:])
                        rc = evp.tile([1, NQT], f32, name="rc", tag="rc")
                        nc.gpsimd.dma_start(out=rc[:], in_=rr[:])
                        rb = evp.tile([64, NQT], f32, name="rb", tag="rb")
                        nc.gpsimd.partition_broadcast(rb[:], rc[:])
                        nc.vector.tensor_mul(at[s * 64:(s + 1) * 64, :],
                                             pocp[0:64, :], rb[:])
                    nc.sync.dma_start(out=ag_in[j][hp][:], in_=at[:])
                    nc.gpsimd.collective_compute(
                        "AllGather", ALU.bypass, replica_groups=rg,
                        ins=[ag_in[j][hp].opt()], outs=[ag_out[j][hp].opt()])

                def interleave(a, b, period=2):
                    """merge list b into list a, one b-item every `period`."""
                    out_, bi = [], 0
                    for k, u in enumerate(a):
                        out_.append(u)
                        if k % period == period - 1 and bi < len(b):
                            out_.append(b[bi])
                            bi += 1
                    out_ += b[bi:]
                    return out_

                class Pacer:
                    """Spread filler steps evenly over a window's chunk
                    slots so no stretch of attention runs bare (exp-bound
                    with an idling, HAM-cooling PE)."""

                    def __init__(self, steps, slots):
                        self.steps, self.slots = steps, max(slots, 1)
                        self.i, self.seen = 0, 0

                    def tick(self):
                        self.seen += 1
                        want = (len(self.steps) * self.seen) // self.slots
                        while self.i < min(want, len(self.steps)):
                            self.steps[self.i]()
                            self.i += 1

                    def drain(self):
                        while self.i < len(self.steps):
                            self.steps[self.i]()
                            self.i += 1

                # ---------------- the schedule ----------------
                for u in proj_steps(0):
                    u()
                for j in range(nj):
                    steps = []
                    if j == 0:
                        steps = proj_steps(1)
                    elif j == 1:
                        steps = proj_steps(2)
                    elif j == 2:
                        steps = interleave(proj_steps(3),
                                           [load_ag(0, m) for m in range(4)] +
                                           [load_ag(1, m) for m in range(4)])
                    else:
                        # window 3 is exp-heavy (~85us) with 64 chunk slots:
                        # all three earlier o_proj tiles fill it.  (2,m) ag
                        # buffers only free up as o_proj(0) retires (0,m).
                        steps = oproj_steps(0)
                        steps += [load_ag(2, m) for m in range(4)]
                        steps += oproj_steps(1)
                        steps += oproj_steps(2)
                        # (3,0)/(3,1) gathers finish mid-window; loading them
                        # here lets tail o_proj m0/m1 fill the wait for the
                        # last gather.  Their sync-queue waits end before the
                        # final at->gather-input DMA is ready, so nothing
                        # downstream blocks.
                        steps.append(load_ag(3, 0))
                        steps.append(load_ag(3, 1))
                    pacer = Pacer(steps, 4 * len(plans[j]))
                    for hp in range(4):
                        attn_hp(j, hp, pacer)
                    pacer.drain()
                # tail: last q-tile's o_proj accumulates m-major so only the
                # final quarter waits on the last (small) AllGather.  ALL of
                # the last tile's gather loads go here — a load emitted
                # mid-window waits on its gather ON THE SYNC QUEUE and
                # head-of-line-blocks the at->gather-input DMAs behind it,
                # cascading the remaining gathers ~25us late.
                jl = nj - 1
                load_ag(jl, 2)()
                load_ag(jl, 3)()
                pf = [pmain.tile([128, QDIM], f32, name="pf", tag="ps")
                      for _ in range(2)] + \
                     [psS.tile([128, QDIM], f32, name="pf2", tag="pss")
                      for _ in range(2)]
                for m in range(4):
                    agm = ag_tiles[(jl, m)]
                    for tt in range(4):
                        for g in range(NGROUPS):
                            nc.tensor.matmul(
                                pf[tt][:],
                                agm[:, g, tt * 128:(tt + 1) * 128],
                                ow_sb[:, g * 4 + m, :],
                                start=(m == 0 and g == 0),
                                stop=(m == 3 and g == NGROUPS - 1))
                for tt in range(4):
                    oproj_evict(jl, tt, pf[tt])

    nc.compile()
    return nc


def prep_inputs(hidden, positions, mask, q_w, q_b, k_w, k_b, v_w, v_b, o_w,
                emt_tiles):
    """Host-side shard + transform -> in_maps for the 8 cores."""
    B, T, _ = hidden.shape
    nhc = HID // 128
    pos = np.asarray(positions)[0].astype(np.float32)
    inv_freq = (1.0 / (THETA ** (np.arange(0, D, 2, dtype=np.float32) / D)))
    freqs = pos[:, None] * inv_freq[None, :]          # (T, 32)
    cos_t, sin_t = np.cos(freqs).T, np.sin(freqs).T   # (32, T)
    cos_tab = np.ascontiguousarray(np.tile(cos_t, (4, 1)), dtype=np.float32)
    ssin_tab = np.ascontiguousarray(
        np.concatenate([sin_t, -sin_t, sin_t, -sin_t], axis=0),
        dtype=np.float32)

    if emt_tiles:
        emt_arr = np.ascontiguousarray(
            np.stack(emt_tiles, axis=1)).astype(BF16)  # [128, n_emt, 512]
    else:
        emt_arr = np.zeros((128, 1, NQT), BF16)

    def chunked(w):   # [HID, O] -> [128, nhc, O]
        return np.ascontiguousarray(
            w.reshape(nhc, 128, w.shape[1]).transpose(1, 0, 2)).astype(BF16)

    nj = T // NQT
    xts = []
    for b in range(B):
        a = np.asarray(hidden[b], np.float32)          # [T, HID]
        a = a.reshape(nj, NQT, nhc, 128)               # [tb, u, c, p]
        xts.append(np.ascontiguousarray(
            a.transpose(0, 3, 2, 1)).astype(BF16))     # [tb, p, c, u]

    in_maps = []
    for c in range(NCORES):
        b, g = c // NGROUPS, c % NGROUPS
        qsl = slice(QDIM * g, QDIM * (g + 1))
        ksl = slice(KVDIM * g, KVDIM * (g + 1))
        in_maps.append({
            "xt": xts[b],
            "wqt": chunked(np.asarray(q_w[qsl, :]).T),
            "wkt": chunked(np.asarray(k_w[ksl, :]).T),
            "wvt": chunked(np.asarray(v_w[ksl, :]).T),
            "qb": np.ascontiguousarray(
                np.asarray(q_b[qsl], np.float32).reshape(4, 128).T),
            "kb": np.ascontiguousarray(
                np.asarray(k_b[ksl], np.float32).reshape(1, 128).T),
            "vb": np.asarray(v_b[ksl]).astype(BF16).reshape(1, KVDIM),
            "cosq": cos_tab,
            "ssin": ssin_tab,
            "emt": emt_arr,
            "owt": chunked(np.asarray(o_w).T[:, qsl]),
        })
    return in_maps


def _ensure_ntff_hook():
    """Provide antenv.axon_hooks in containers whose antenv stub lacks it,
    wiring the ctypes NTFF profiler from the injected axon boot package."""
    import sys
    import types
    try:
        from antenv.axon_hooks import get_axon_ntff_profile_hook  # noqa: F401
        return True
    except ImportError:
        pass
    try:
        import antenv
        from trn_agent_boot.trn_boot import _ntff_profile_via_ctypes
        hook = _ntff_profile_via_ctypes("/opt/axon/libaxon_pjrt.so")
        if hook is None:
            return False
        mod = types.ModuleType("antenv.axon_hooks")
        state = {"h": hook}
        mod.get_axon_ntff_profile_hook = lambda: state["h"]
        mod.set_axon_ntff_profile_hook = lambda h: state.__setitem__("h", h)
        sys.modules["antenv.axon_hooks"] = mod
        antenv.axon_hooks = mod
        return True
    except Exception:
        return False


def kernel(hidden, positions, mask, q_w, q_b, k_w, k_b, v_w, v_b, o_w):
    global LAST_RESULT
    from concourse import bass_utils

    hidden = np.asarray(hidden)
    B, T, _ = hidden.shape
    mask_key = (T, hash(np.asarray(mask).tobytes()))
    if mask_key not in _cache:
        plans, emt_tiles = plan_mask(mask, T)
        nc = build_graph(T, plans, len(emt_tiles))
        _cache[mask_key] = (nc, emt_tiles)
    nc, emt_tiles = _cache[mask_key]

    in_maps = prep_inputs(hidden, positions, mask, q_w, q_b, k_w, k_b,
                          v_w, v_b, o_w, emt_tiles)
    trace = os.environ.get("BASS_KERNEL_TRACE", "0") == "1"
    if trace:
        trace = _ensure_ntff_hook()
    res = bass_utils.run_bass_kernel_spmd(nc, in_maps,
                                          core_ids=list(range(NCORES)),
                                          trace=trace)
    LAST_RESULT = res
    out = np.zeros((B, T, HID), np.float32)
    for c in range(NCORES):
        b, g = c // NGROUPS, c % NGROUPS
        out[b, :, QDIM * g:QDIM * (g + 1)] = \
            res.results[c]["out"].astype(np.float32)
    return out


# revision 27
# speedup vs baseline: 1.0169x; 1.0017x over previous
"""GQA attention (32 q heads / 8 kv heads, D=64, HID=2048, B=2, T=2048)
distributed over 8 TRN2 NeuronCores.

Sharding: 2-way data parallel (batch) x 4-way tensor parallel (head groups).
Core c handles batch c//4 and head group g=c%4 (q heads [8g,8g+8), kv heads
[2g,2g+2)).  Each core projects Q^T/K^T (transposed layout: head-dims on
partitions, T on free axis), computes V^T the same cheap way (weights
stationary, N=512 streams) and PE-transposes it into the [keys, dims] layout
that P@V needs.  Scores^T = K @ Q^T per head with keys on partitions, exp via
ScalarE (no max-subtraction needed at these magnitudes; masked entries
multiply to exactly 0 by a host-precomputed exp(mask) factor), then
out^T = Vext^T @ P^T where Vext carries a ones column producing the softmax
denominators for free.

The attention phase is ScalarE(exp)-bound (~1.33us per key-chunk vs ~0.78us
of matmul), so all other PE work — next block's projections, o_proj of
previous q-tiles, gather loads — is chopped into ~1us micro-steps and
interleaved ONE PER KEY-CHUNK into the attention emission, with the score
matmuls software-pipelined one chunk ahead so the exp stream never waits.
Attention outputs are AllGathered per head-pair (16 gathers of 128KB); a
dummy gather at t=0 absorbs the collective warm-up, and the last q-tile's
o_proj accumulates m-major so only its final quarter waits on the last
gather.  All host-side layouts give every DMA >=4KB contiguous per partition
(hardware DGE fast path).
"""

import os
import numpy as np
import ml_dtypes

BF16 = ml_dtypes.bfloat16

HQ, HKV, D, HID, THETA = 32, 8, 64, 2048, 10000.0
NCORES, NGROUPS = 8, 4
QDIM = HQ * D // NGROUPS        # 512 q dims per core
KVDIM = HKV * D // NGROUPS      # 128 kv dims per core
NQT = 512                       # query tile (free dim per PSUM bank)
NKC = 128                       # key chunk (partition dim)

_cache = {}
LAST_RESULT = None              # BassKernelResults of the most recent run


def plan_mask(mask, T):
    """Classify (key-chunk i, q-tile j) tiles of exp(mask).T.

    Returns (plans, emt_tiles): plans[j] = list of (i, kind, emt_idx) where
    kind 0 = no mask needed (exp(mask)==1 on tile), kind 1 = multiply by
    emt_tiles[emt_idx].  All-zero tiles are skipped entirely (they contribute
    nothing to P@V nor to the softmax denominator).
    """
    m = np.asarray(mask, dtype=np.float32).reshape(T, T)
    with np.errstate(under="ignore"):
        em = np.exp(m).T.astype(np.float32)   # em[k, q] = exp(mask[q, k])
    nj, nk = T // NQT, T // NKC
    plans, emt_tiles = [], []
    for j in range(nj):
        pj = []
        for i in range(nk):
            t = em[i * NKC:(i + 1) * NKC, j * NQT:(j + 1) * NQT]
            if not t.any():
                continue
            if (t == 1.0).all():
                pj.append((i, 0, -1))
            else:
                pj.append((i, 1, len(emt_tiles)))
                emt_tiles.append(t.astype(BF16))
        plans.append(pj)
    return plans, emt_tiles


def build_graph(T, plans, n_emt):
    """Build the SPMD Bacc graph (same on all 8 cores; shards arrive as data)."""
    import concourse.bass as bass  # noqa: F401
    import concourse.mybir as mybir
    import concourse.tile as tile
    from concourse import bacc, masks

    f32, bf16 = mybir.dt.float32, mybir.dt.bfloat16
    AF, ALU = mybir.ActivationFunctionType, mybir.AluOpType

    nj = T // NQT          # q tiles
    nhc = HID // 128       # contraction chunks over hidden dim (16)
    noc = (HQ * D) // 128  # contraction chunks over gathered head dim (16)
    nem = max(n_emt, 1)
    assert nj == 4, "schedule below is specialized for T=2048"

    nc = bacc.Bacc("TRN2", target_bir_lowering=False, debug=False,
                   num_devices=NCORES)

    # host-prepped layouts: every tensor reads contiguous >=4KB per partition
    xt = nc.dram_tensor("xt", [nj, 128, nhc, NQT], bf16,
                        kind="ExternalInput").ap()
    wqt = nc.dram_tensor("wqt", [128, nhc, QDIM], bf16,
                         kind="ExternalInput").ap()
    wkt = nc.dram_tensor("wkt", [128, nhc, KVDIM], bf16,
                         kind="ExternalInput").ap()
    wvt = nc.dram_tensor("wvt", [128, nhc, KVDIM], bf16,
                         kind="ExternalInput").ap()
    qb = nc.dram_tensor("qb", [128, 4], f32, kind="ExternalInput").ap()
    kb = nc.dram_tensor("kb", [128, 1], f32, kind="ExternalInput").ap()
    vb = nc.dram_tensor("vb", [1, KVDIM], bf16, kind="ExternalInput").ap()
    cosq = nc.dram_tensor("cosq", [128, T], f32, kind="ExternalInput").ap()
    ssin = nc.dram_tensor("ssin", [128, T], f32, kind="ExternalInput").ap()
    emt = nc.dram_tensor("emt", [128, nem, NQT], bf16,
                         kind="ExternalInput").ap()
    owt = nc.dram_tensor("owt", [128, noc, QDIM], bf16,
                         kind="ExternalInput").ap()
    out = nc.dram_tensor("out", [T, QDIM], bf16, kind="ExternalOutput").ap()

    rg = [[0, 1, 2, 3], [4, 5, 6, 7]]

    with tile.TileContext(nc) as tc:
        with tc.tile_pool(name="dramp", bufs=1, space="DRAM") as dramp:
            ag_in = [[dramp.tile([128, NQT], bf16, name=f"agin{j}_{m}")
                      for m in range(4)] for j in range(nj)]
            ag_out = [[dramp.tile([NGROUPS * 128, NQT], bf16,
                                  name=f"agout{j}_{m}")
                       for m in range(4)] for j in range(nj)]
            dum_in = dramp.tile([1, 128], bf16, name="dum_in")
            dum_out = dramp.tile([4, 128], bf16, name="dum_out")
            agl_in = [dramp.tile([64, NQT], bf16, name=f"aglin{h}")
                      for h in (0, 1)]
            agl_out = [dramp.tile([NGROUPS * 64, NQT], bf16,
                                  name=f"aglout{h}") for h in (0, 1)]

        with tc.tile_pool(name="persist", bufs=1) as pp:
            # Q^T per head-pair chunk: [128 (2 heads x 64), T]
            qt = [pp.tile([128, T], bf16, name=f"qt{m}") for m in range(4)]
            # K^T duplicated per kv head: [128 = kv dup'd twice, T]
            ktd = [pp.tile([128, T], bf16, name=f"ktd{k}") for k in range(2)]
            # V per key chunk: [128 keys, 130] (V0|one|V1|one)
            vsb = [pp.tile([128, 130], bf16, name=f"v{i}")
                   for i in range(T // NKC)]
            ow_sb = pp.tile([128, noc, QDIM], bf16, name="ow_sb")
            wqq = [pp.tile([128, 4, QDIM], bf16, name=f"wqq{r}")
                   for r in range(4)]
            wk_sb = pp.tile([128, nhc, KVDIM], bf16, name="wk_sb")
            wv_sb = pp.tile([128, nhc, KVDIM], bf16, name="wv_sb")
            cos_sb = pp.tile([128, T], f32, name="cos_sb")
            ssin_sb = pp.tile([128, T], f32, name="ssin_sb")
            emt_sb = pp.tile([128, nem, NQT], bf16, name="emt_sb")
            ident = pp.tile([128, 128], bf16, name="ident")
            vb_sb = pp.tile([1, KVDIM], bf16, name="vb_sb")
            ones_row = pp.tile([1, NQT], bf16, name="ones_row")
            qb_sb = pp.tile([128, 4], f32, name="qb_sb")
            kb_sb = pp.tile([128, 1], f32, name="kb_sb")

            nc.sync.dma_start(out=qb_sb[:], in_=qb)
            nc.sync.dma_start(out=kb_sb[:], in_=kb)
            nc.sync.dma_start(out=vb_sb[:], in_=vb)
            nc.vector.memset(ones_row[:], 1.0)
            # touch Exp once so the ACT table load (~2.7us) happens during
            # the startup DMA wait, not before the first real softmax
            warm = pp.tile([1, 2], bf16, name="warm")
            nc.scalar.activation(warm[:], ones_row[0:1, 0:2], AF.Exp,
                                 scale=0.125)
            # dummy gather: absorbs the one-time collective barrier/warm-up
            # (~12us trigger delay + ~2.5x duration) before the first real one
            nc.sync.dma_start(out=dum_in[:], in_=ones_row[0:1, 0:128])
            nc.gpsimd.collective_compute(
                "AllGather", ALU.bypass, replica_groups=rg,
                ins=[dum_in.opt()], outs=[dum_out.opt()])
            masks.make_identity(nc, ident[:])

            with tc.tile_pool(name="projx", bufs=2) as px, \
                 tc.tile_pool(name="projtmp", bufs=2) as ptmp, \
                 tc.tile_pool(name="pmain", bufs=2, space="PSUM") as pmain, \
                 tc.tile_pool(name="psS", bufs=2, space="PSUM") as psS, \
                 tc.tile_pool(name="psO", bufs=1, space="PSUM") as psO, \
                 tc.tile_pool(name="ptp", bufs=3) as ptp, \
                 tc.tile_pool(name="evp", bufs=2) as evp, \
                 tc.tile_pool(name="agp", bufs=8) as agp, \
                 tc.tile_pool(name="outp", bufs=2) as outp:

                x_tiles = {}

                def load_x_quarter(tb, qr):
                    xq = px.tile([128, 4, NQT], bf16, name="x_sb", tag="x_sb",
                                 bufs=8)
                    nc.sync.dma_start(out=xq[:],
                                      in_=xt[tb, :, qr * 4:(qr + 1) * 4, :])
                    x_tiles.setdefault(tb, []).append(xq)

                # startup: first x/wq quarter (first matmuls at ~1MB), then
                # cos/sin (they gate the RoPE evict chain and with it the
                # PSUM-pool rotation of the whole projection), then the rest.
                load_x_quarter(0, 0)
                nc.sync.dma_start(out=wqq[0][:], in_=wqt[:, 0:4, :])
                nc.sync.dma_start(out=cos_sb[:], in_=cosq)
                nc.sync.dma_start(out=ssin_sb[:], in_=ssin)
                for qr in range(1, 4):
                    load_x_quarter(0, qr)
                    nc.sync.dma_start(out=wqq[qr][:],
                                      in_=wqt[:, qr * 4:(qr + 1) * 4, :])
                nc.sync.dma_start(out=wk_sb[:], in_=wkt)
                nc.sync.dma_start(out=wv_sb[:], in_=wvt)
                nc.sync.dma_start(out=emt_sb[:], in_=emt)
                nc.sync.dma_start(out=ow_sb[:], in_=owt)

                def rope_evict(ps, bias_col, dst, ts):
                    """dst = RoPE(ps + bias) cast to bf16."""
                    t2 = ptmp.tile([128, NQT], f32, name="t2", tag="t2")
                    nc.vector.scalar_tensor_tensor(
                        t2[:], ps[:], bias_col, ssin_sb[:, ts],
                        op0=ALU.add, op1=ALU.mult)
                    t2s = ptmp.tile([128, NQT], f32, name="t2s", tag="t2s")
                    for blk in range(4):
                        sb = blk ^ 1
                        # off the sync queue: keeps it free for attention
                        nc.gpsimd.dma_start(
                            out=t2s[blk * 32:(blk + 1) * 32, :],
                            in_=t2[sb * 32:(sb + 1) * 32, :])
                    t1 = ptmp.tile([128, NQT], f32, name="t1", tag="t1")
                    nc.vector.scalar_tensor_tensor(
                        t1[:], ps[:], bias_col, cos_sb[:, ts],
                        op0=ALU.add, op1=ALU.mult)
                    nc.vector.tensor_add(dst, t1[:], t2s[:])

                def proj_steps(tb):
                    """Projection of T-block tb as ~1us micro-steps."""
                    ts = slice(tb * NQT, (tb + 1) * NQT)
                    state = {}

                    def qs(m, qr):
                        def f():
                            xq = x_tiles[tb]
                            if qr == 0:
                                state[m] = pmain.tile([128, NQT], f32,
                                                      name="ps", tag="ps")
                            ps = state[m]
                            for c in range(qr * 4, qr * 4 + 4):
                                nc.tensor.matmul(
                                    ps[:],
                                    wqq[c // 4][:, c % 4,
                                                m * 128:(m + 1) * 128],
                                    xq[c // 4][:, c % 4, :],
                                    start=(c == 0), stop=(c == nhc - 1))
                            if qr == 3:
                                rope_evict(ps, qb_sb[:, m:m + 1],
                                           qt[m][:, ts], ts)
                        return f

                    def ks(qr):
                        def f():
                            xq = x_tiles[tb]
                            if qr == 0:
                                state['k'] = pmain.tile([128, NQT], f32,
                                                        name="psk", tag="ps")
                            psk = state['k']
                            for c in range(qr * 4, qr * 4 + 4):
                                nc.tensor.matmul(psk[:], wk_sb[:, c, :],
                                                 xq[c // 4][:, c % 4, :],
                                                 start=(c == 0),
                                                 stop=(c == nhc - 1))
                            if qr == 3:
                                kf = ptmp.tile([128, NQT], bf16, name="kf",
                                               tag="kf")
                                rope_evict(psk, kb_sb[:, 0:1], kf[:], ts)
                                for half in (0, 1):
                                    for dsth in (0, 1):
                                        nc.gpsimd.dma_start(
                                            out=ktd[half][dsth * 64:
                                                          (dsth + 1) * 64, ts],
                                            in_=kf[half * 64:(half + 1) * 64,
                                                   :])
                        return f

                    def vs(qr):
                        # V^T: kv dims on partitions, T on free — weights
                        # stationary, N=512 streams.
                        def f():
                            xq = x_tiles[tb]
                            if qr == 0:
                                state['v'] = pmain.tile([128, NQT], f32,
                                                        name="psv", tag="ps")
                            psv = state['v']
                            for c in range(qr * 4, qr * 4 + 4):
                                nc.tensor.matmul(psv[:], wv_sb[:, c, :],
                                                 xq[c // 4][:, c % 4, :],
                                                 start=(c == 0), stop=False)
                            if qr == 3:
                                nc.tensor.matmul(psv[:], vb_sb[:],
                                                 ones_row[:],
                                                 start=False, stop=True)
                                vt = ptmp.tile([128, NQT], bf16, name="vt",
                                               tag="vt")
                                nc.vector.tensor_copy(vt[:], psv[:])
                                state['vt'] = vt
                        return f

                    def vtr():
                        # PE-transpose V^T back to [keys, dims] for P@V.
                        vt = state['vt']
                        pstr = pmain.tile([128, NQT], bf16, name="pstr",
                                          tag="ps")
                        for ti in range(4):
                            nc.tensor.transpose(
                                pstr[:, ti * 128:(ti + 1) * 128],
                                vt[:, ti * 128:(ti + 1) * 128], ident[:])
                        for ti in range(4):
                            vi = tb * 4 + ti
                            nc.vector.memset(vsb[vi][:, 64:65], 1.0)
                            nc.vector.memset(vsb[vi][:, 129:130], 1.0)
                            nc.vector.tensor_copy(
                                vsb[vi][:, 0:64],
                                pstr[:, ti * 128:ti * 128 + 64])
                            nc.vector.tensor_copy(
                                vsb[vi][:, 65:129],
                                pstr[:, ti * 128 + 64:(ti + 1) * 128])

                    # q0/K/V first: attention(tb) head-pair 0 needs exactly
                    # these, so its score->exp stream starts ~20us earlier
                    # than with the m-ordered emission.
                    steps = [lambda qr=qr: load_x_quarter(tb, qr)
                             for qr in range(4)] if tb > 0 else []
                    steps += [qs(0, qr) for qr in range(4)]
                    steps += [ks(qr) for qr in range(4)]
                    steps += [vs(qr) for qr in range(4)]
                    steps.append(vtr)
                    for m in range(1, 4):
                        steps += [qs(m, qr) for qr in range(4)]
                    return steps

                ag_tiles = {}

                def load_ag(j, m):
                    def f():
                        ag_sbm = agp.tile([128, NGROUPS, NQT], bf16,
                                          name="ag_sb")
                        if j == nj - 1 and m == 3:
                            for h in (0, 1):
                                nc.sync.dma_start(
                                    out=ag_sbm[h * 64:(h + 1) * 64, :, :],
                                    in_=agl_out[h].rearrange(
                                        "(g p) t -> p g t", p=64))
                        else:
                            nc.sync.dma_start(
                                out=ag_sbm[:],
                                in_=ag_out[j][m].rearrange("(g p) t -> p g t",
                                                           p=128))
                        ag_tiles[(j, m)] = ag_sbm
                    return f

                def oproj_evict(j, tt, pf):
                    ot = outp.tile([128, QDIM], bf16, name="ot", tag="ot")
                    nc.vector.tensor_copy(ot[:], pf[:])
                    nc.sync.dma_start(
                        out=out[j * NQT + tt * 128:
                                j * NQT + (tt + 1) * 128, :],
                        in_=ot[:])

                def oproj_steps(j):
                    state = {}

                    def os(tt, m):
                        def f():
                            if m == 0:
                                state[tt] = pmain.tile([128, QDIM], f32,
                                                       name="pf", tag="ps")
                            pf = state[tt]
                            agm = ag_tiles[(j, m)]
                            for g in range(NGROUPS):
                                nc.tensor.matmul(
                                    pf[:],
                                    agm[:, g, tt * 128:(tt + 1) * 128],
                                    ow_sb[:, g * 4 + m, :],
                                    start=(m == 0 and g == 0),
                                    stop=(m == 3 and g == NGROUPS - 1))
                            if m == 3:
                                oproj_evict(j, tt, pf)
                        return f

                    return [os(tt, m) for tt in range(4) for m in range(4)]

                def attn_hp(j, hp, filler):
                    qs_ = slice(j * NQT, (j + 1) * NQT)
                    kv = hp // 2
                    po0 = psO.tile([65, NQT], f32, name="po0", tag="po0")
                    po1 = psO.tile([65, NQT], f32, name="po1", tag="po1")
                    ch = plans[j]
                    n_ch = len(ch)
                    pss_t = {}

                    def scores(ci):
                        i = ch[ci][0]
                        pss = psS.tile([128, 1024], f32, name="pss",
                                       tag="pss")
                        # head-lo on array rows 0:64, head-hi on 64:128 —
                        # concurrent row-groups, separate PSUM banks
                        nc.tensor.matmul(
                            pss[:, 0:512],
                            ktd[kv][0:64, i * NKC:(i + 1) * NKC],
                            qt[hp][0:64, qs_], start=True, stop=True)
                        nc.tensor.matmul(
                            pss[:, 512:1024],
                            ktd[kv][64:128, i * NKC:(i + 1) * NKC],
                            qt[hp][64:128, qs_], start=True, stop=True)
                        pss_t[ci] = pss

                    scores(0)
                    for ci in range(n_ch):
                        if ci + 1 < n_ch:
                            scores(ci + 1)
                        i, kind, gi = ch[ci]
                        pss = pss_t.pop(ci)
                        pt = ptp.tile([128, 1024], bf16, name="pt", tag="pt")
                        nc.scalar.activation(pt[:], pss[:], AF.Exp,
                                             scale=0.125)
                        if kind == 1:
                            nc.vector.tensor_mul(pt[:, 0:512], pt[:, 0:512],
                                                 emt_sb[:, gi, :])
                            nc.vector.tensor_mul(pt[:, 512:1024],
                                                 pt[:, 512:1024],
                                                 emt_sb[:, gi, :])
                        # ~1us micro-steps of other PE work, emitted BETWEEN
                        # this chunk's scores and its P@V: the PE queue never
                        # head-of-line-stalls on the exp wait, which both
                        # fills the gap and keeps the HAM activity monitor
                        # seeing a busy PE (K=8/8, full clock).  Double pull
                        # on the first chunk: covers the previous head-pair's
                        # softmax-evict chain (the new po accumulators wait
                        # on its PSUM copies).
                        filler.tick()
                        if ci == 0:
                            filler.tick()
                            filler.tick()
                        vsl = (vsb[i][:, 0:65] if kv == 0
                               else vsb[i][:, 65:130])
                        nc.tensor.matmul(po0[:], vsl, pt[:, 0:512],
                                         start=(ci == 0),
                                         stop=(ci == n_ch - 1))
                        nc.tensor.matmul(po1[:], vsl, pt[:, 512:1024],
                                         start=(ci == 0),
                                         stop=(ci == n_ch - 1))
                    at = evp.tile([128, NQT], bf16, name="at", tag="at",
                                  bufs=2)
                    split = (j == nj - 1 and hp == 3)
                    for s, po in enumerate((po0, po1)):
                        # One fast copy frees the PSUM bank; the divide chain
                        # then runs off the PE critical path from SBUF.  The
                        # two heads' copies run on different engines (and
                        # different PSUM banks): ScalarE is idle between the
                        # hps' exp streams, so the next hp's accumulators
                        # free up ~1us sooner.
                        pocp = evp.tile([65, NQT], f32, name="pocp",
                                        tag=f"pocp{s}", bufs=1)
                        if s == 0:
                            nc.scalar.copy(pocp[:], po[:])
                        else:
                            nc.vector.tensor_copy(pocp[:], po[:])
                        # reciprocal is ~6 cycles/elem serial per partition:
                        # spread the 512 sums over 128 lanes via two small
                        # DMAs so it costs ~0.2us instead of 1.7us
                        rs = evp.tile([128, 4], f32, name="rs", tag="rs")
                        nc.gpsimd.dma_start(out=rs[:], in_=pocp[64:65, :])
                        rr = evp.tile([128, 4], f32, name="rr", tag="rr")
                        nc.vector.reciprocal(rr[:], rs[:])
                        rc = evp.tile([1, NQT], f32, name="rc", tag="rc")
                        nc.gpsimd.dma_start(out=rc[:], in_=rr[:])
                        rb = evp.tile([64, NQT], f32, name="rb", tag="rb")
                        nc.gpsimd.partition_broadcast(rb[:], rc[:])
                        nc.vector.tensor_mul(at[s * 64:(s + 1) * 64, :],
                                             pocp[0:64, :], rb[:])
                        if split:
                            # the very last gather goes out in two halves,
                            # each launched as soon as its head's divide is
                            # done: the tail waits on a ~64KB gather instead
                            # of a 128KB one.
                            nc.sync.dma_start(out=agl_in[s][:],
                                              in_=at[s * 64:(s + 1) * 64, :])
                            nc.gpsimd.collective_compute(
                                "AllGather", ALU.bypass, replica_groups=rg,
                                ins=[agl_in[s].opt()],
                                outs=[agl_out[s].opt()])
                    if not split:
                        nc.sync.dma_start(out=ag_in[j][hp][:], in_=at[:])
                        nc.gpsimd.collective_compute(
                            "AllGather", ALU.bypass, replica_groups=rg,
                            ins=[ag_in[j][hp].opt()], outs=[ag_out[j][hp].opt()])

                def interleave(a, b, period=2):
                    """merge list b into list a, one b-item every `period`."""
                    out_, bi = [], 0
                    for k, u in enumerate(a):
                        out_.append(u)
                        if k % period == period - 1 and bi < len(b):
                            out_.append(b[bi])
                            bi += 1
                    out_ += b[bi:]
                    return out_

                class Pacer:
                    """Spread filler steps evenly over a window's chunk
                    slots so no stretch of attention runs bare (exp-bound
                    with an idling, HAM-cooling PE)."""

                    def __init__(self, steps, slots):
                        self.steps, self.slots = steps, max(slots, 1)
                        self.i, self.seen = 0, 0

                    def tick(self):
                        self.seen += 1
                        want = (len(self.steps) * self.seen) // self.slots
                        while self.i < min(want, len(self.steps)):
                            self.steps[self.i]()
                            self.i += 1

                    def drain(self):
                        while self.i < len(self.steps):
                            self.steps[self.i]()
                            self.i += 1

                # ---------------- the schedule ----------------
                for u in proj_steps(0):
                    u()
                for j in range(nj):
                    steps = []
                    if j == 0:
                        steps = proj_steps(1)
                    elif j == 1:
                        steps = proj_steps(2)
                    elif j == 2:
                        steps = interleave(proj_steps(3),
                                           [load_ag(0, m) for m in range(4)] +
                                           [load_ag(1, m) for m in range(4)])
                    else:
                        # window 3 is exp-heavy (~85us) with 64 chunk slots:
                        # all three earlier o_proj tiles fill it.  (2,m) ag
                        # buffers only free up as o_proj(0) retires (0,m).
                        steps = oproj_steps(0)
                        steps += [load_ag(2, m) for m in range(4)]
                        steps += oproj_steps(1)
                        steps += oproj_steps(2)
                        # (3,0)/(3,1) gathers finish mid-window; loading them
                        # here lets tail o_proj m0/m1 fill the wait for the
                        # last gather.  Their sync-queue waits end before the
                        # final at->gather-input DMA is ready, so nothing
                        # downstream blocks.
                        steps.append(load_ag(3, 0))
                        steps.append(load_ag(3, 1))
                    pacer = Pacer(steps, 4 * len(plans[j]))
                    for hp in range(4):
                        attn_hp(j, hp, pacer)
                    pacer.drain()
                # tail: last q-tile's o_proj accumulates m-major so only the
                # final quarter waits on the last (small) AllGather.  ALL of
                # the last tile's gather loads go here — a load emitted
                # mid-window waits on its gather ON THE SYNC QUEUE and
                # head-of-line-blocks the at->gather-input DMAs behind it,
                # cascading the remaining gathers ~25us late.
                jl = nj - 1
                load_ag(jl, 2)()
                load_ag(jl, 3)()
                pf = [pmain.tile([128, QDIM], f32, name="pf", tag="ps")
                      for _ in range(2)] + \
                     [psS.tile([128, QDIM], f32, name="pf2", tag="pss")
                      for _ in range(2)]
                for m in range(4):
                    agm = ag_tiles[(jl, m)]
                    for tt in range(4):
                        for g in range(NGROUPS):
                            nc.tensor.matmul(
                                pf[tt][:],
                                agm[:, g, tt * 128:(tt + 1) * 128],
                                ow_sb[:, g * 4 + m, :],
                                start=(m == 0 and g == 0),
                                stop=(m == 3 and g == NGROUPS - 1))
                for tt in range(4):
                    oproj_evict(jl, tt, pf[tt])

    nc.compile()
    return nc


def prep_inputs(hidden, positions, mask, q_w, q_b, k_w, k_b, v_w, v_b, o_w,
                emt_tiles):
    """Host-side shard + transform -> in_maps for the 8 cores."""
    B, T, _ = hidden.shape
    nhc = HID // 128
    pos = np.asarray(positions)[0].astype(np.float32)
    inv_freq = (1.0 / (THETA ** (np.arange(0, D, 2, dtype=np.float32) / D)))
    freqs = pos[:, None] * inv_freq[None, :]          # (T, 32)
    cos_t, sin_t = np.cos(freqs).T, np.sin(freqs).T   # (32, T)
    cos_tab = np.ascontiguousarray(np.tile(cos_t, (4, 1)), dtype=np.float32)
    ssin_tab = np.ascontiguousarray(
        np.concatenate([sin_t, -sin_t, sin_t, -sin_t], axis=0),
        dtype=np.float32)

    if emt_tiles:
        emt_arr = np.ascontiguousarray(
            np.stack(emt_tiles, axis=1)).astype(BF16)  # [128, n_emt, 512]
    else:
        emt_arr = np.zeros((128, 1, NQT), BF16)

    def chunked(w):   # [HID, O] -> [128, nhc, O]
        return np.ascontiguousarray(
            w.reshape(nhc, 128, w.shape[1]).transpose(1, 0, 2)).astype(BF16)

    nj = T // NQT
    xts = []
    for b in range(B):
        a = np.asarray(hidden[b], np.float32)          # [T, HID]
        a = a.reshape(nj, NQT, nhc, 128)               # [tb, u, c, p]
        xts.append(np.ascontiguousarray(
            a.transpose(0, 3, 2, 1)).astype(BF16))     # [tb, p, c, u]

    in_maps = []
    for c in range(NCORES):
        b, g = c // NGROUPS, c % NGROUPS
        qsl = slice(QDIM * g, QDIM * (g + 1))
        ksl = slice(KVDIM * g, KVDIM * (g + 1))
        in_maps.append({
            "xt": xts[b],
            "wqt": chunked(np.asarray(q_w[qsl, :]).T),
            "wkt": chunked(np.asarray(k_w[ksl, :]).T),
            "wvt": chunked(np.asarray(v_w[ksl, :]).T),
            "qb": np.ascontiguousarray(
                np.asarray(q_b[qsl], np.float32).reshape(4, 128).T),
            "kb": np.ascontiguousarray(
                np.asarray(k_b[ksl], np.float32).reshape(1, 128).T),
            "vb": np.asarray(v_b[ksl]).astype(BF16).reshape(1, KVDIM),
            "cosq": cos_tab,
            "ssin": ssin_tab,
            "emt": emt_arr,
            "owt": chunked(np.asarray(o_w).T[:, qsl]),
        })
    return in_maps


def _ensure_ntff_hook():
    """Provide antenv.axon_hooks in containers whose antenv stub lacks it,
    wiring the ctypes NTFF profiler from the injected axon boot package."""
    import sys
    import types
    try:
        from antenv.axon_hooks import get_axon_ntff_profile_hook  # noqa: F401
        return True
    except ImportError:
        pass
    try:
        import antenv
        from trn_agent_boot.trn_boot import _ntff_profile_via_ctypes
        hook = _ntff_profile_via_ctypes("/opt/axon/libaxon_pjrt.so")
        if hook is None:
            return False
        mod = types.ModuleType("antenv.axon_hooks")
        state = {"h": hook}
        mod.get_axon_ntff_profile_hook = lambda: state["h"]
        mod.set_axon_ntff_profile_hook = lambda h: state.__setitem__("h", h)
        sys.modules["antenv.axon_hooks"] = mod
        antenv.axon_hooks = mod
        return True
    except Exception:
        return False


def kernel(hidden, positions, mask, q_w, q_b, k_w, k_b, v_w, v_b, o_w):
    global LAST_RESULT
    from concourse import bass_utils

    hidden = np.asarray(hidden)
    B, T, _ = hidden.shape
    mask_key = (T, hash(np.asarray(mask).tobytes()))
    if mask_key not in _cache:
        plans, emt_tiles = plan_mask(mask, T)
        nc = build_graph(T, plans, len(emt_tiles))
        _cache[mask_key] = (nc, emt_tiles)
    nc, emt_tiles = _cache[mask_key]

    in_maps = prep_inputs(hidden, positions, mask, q_w, q_b, k_w, k_b,
                          v_w, v_b, o_w, emt_tiles)
    trace = os.environ.get("BASS_KERNEL_TRACE", "0") == "1"
    if trace:
        trace = _ensure_ntff_hook()
    res = bass_utils.run_bass_kernel_spmd(nc, in_maps,
                                          core_ids=list(range(NCORES)),
                                          trace=trace)
    LAST_RESULT = res
    out = np.zeros((B, T, HID), np.float32)
    for c in range(NCORES):
        b, g = c // NGROUPS, c % NGROUPS
        out[b, :, QDIM * g:QDIM * (g + 1)] = \
            res.results[c]["out"].astype(np.float32)
    return out
